# revision 6
# baseline (speedup 1.0000x reference)
"""Trainium2 Bass kernel for nn_BaseModel_14499809591724 (GNN message passing).

Strategy (8 NeuronCores, data-parallel over graph batches):
  - Nodes are split into 8 contiguous shards at graph boundaries (batch is
    sorted), padded to S=6400 rows each; full node table = [8*S, 128] bf16.
  - Each core owns the edges whose dst falls in its shard. Edges are sorted by
    (dst window of 128 nodes, src-table chunk) and chunked into groups of 128.
  - Per GCN conv: batched dma_gather of h[src] rows (WB windows per call) from
    the replicated DRAM table; scaled one-hots for a whole (window, chunk) are
    built with TWO wide DVE tensor_tensor ops using broadcast (stride-0) APs;
    scatter-reduce via PE matmul with the GATHERED rows stationary, producing
    feat-major agg directly (no transpose); self-loop term is one extra matmul
    against a precomputed diag(dinv2) block; then W + bias + ReLU.
  - After each conv that feeds another conv, the 8 local shards are AllGathered
    (bf16, 2 chunks for progressive overlap) to rebuild the replicated table.
  - JumpingKnowledge + per-graph pooling (one-hot matmul) + BN + MLP head +
    softmax run per core on its own 64 graphs; host concatenates 8 x [64, 10].

All heavy compute runs on device. Host does index/layout preprocessing and
edge-weight normalization (deg/dinv/norm), which is sharding metadata.
"""
import sys
import numpy as np
import ml_dtypes

sys.path.insert(0, "/opt/trn_rl_repo")

from concourse import bacc, tile, mybir  # noqa: E402
from concourse.bass_utils import run_bass_kernel_spmd  # noqa: E402

# ---- model / sharding constants (shapes fixed by the problem) ----
NC = 8
N_NODES = 50000
N_EDGES = 800000
F = 128
B = 512
GPC = B // NC          # graphs per core = 64
S = 6400               # padded nodes per shard (max real shard is 6368)
NW = S // 128          # 50 windows per core
TAB = NC * S           # 51200 table rows
NCH = 2                # table chunks (progressive AllGather pipeline)
CHS = S // NCH         # 3200 shard rows per chunk
CHROWS = NC * CHS      # 25600 table rows per chunk (int16-safe)
CH = 9                 # 128-edge groups per (window, table-chunk); max seen 1112
CPW = NCH * CH         # 18 one-hot columns per window
WB = 5                 # windows per dma_gather batch
NBAT = NW // WB        # 10 gather batches per (conv, chunk)
NIDX = WB * CH * 128   # 5760 idxs per gather
ICOLS = NIDX // 16     # 360 wrapped idx columns per gather
NB = 3
BN_EPS = 1e-5

f32 = mybir.dt.float32
bf16 = mybir.dt.bfloat16
i16 = mybir.dt.int16

_PROGRAM = None
import os
REPEAT = int(os.environ.get("REPEAT", "1"))


def _wrap_idxs(runs: np.ndarray) -> np.ndarray:
    """[R, NIDX] int -> [128, R*ICOLS] int16 (16-partition wrap, 8x replicated)."""
    r = runs.shape[0]
    w = runs.reshape(r, -1, 16).transpose(2, 0, 1).reshape(16, -1)
    return np.tile(w.astype(np.int16), (8, 1))


def _preprocess(inp: dict):
    batch = np.asarray(inp["batch"])
    ei = np.asarray(inp["edge_index"])
    ew = np.asarray(inp["edge_attr"], dtype=np.float32)
    x = np.asarray(inp["x"], dtype=np.float32)
    src, dst = ei[0].astype(np.int64), ei[1].astype(np.int64)

    bounds = np.searchsorted(batch, np.arange(0, B + 1, GPC)).astype(np.int64)
    sizes = np.diff(bounds)
    assert sizes.max() <= S, f"shard overflow: {sizes.max()} > {S}"

    node = np.arange(N_NODES, dtype=np.int64)
    core_of = (np.searchsorted(bounds, node, side="right") - 1).astype(np.int64)
    off = node - bounds[core_of]
    # chunk-major table: row = chunk*CHROWS + core*CHS + (off % CHS)
    tab = (off // CHS) * CHROWS + core_of * CHS + (off % CHS)

    deg = (np.bincount(dst, weights=ew.astype(np.float64), minlength=N_NODES) + 1.0)
    deg = deg.astype(np.float32)
    dinv = 1.0 / np.sqrt(deg)
    norm = (dinv[src] * ew * dinv[dst]).astype(np.float32)
    dinv2 = (1.0 / deg).astype(np.float32)

    # full replicated x table (node-major, bf16)
    xtab = np.zeros((TAB, F), dtype=ml_dtypes.bfloat16)
    xtab[tab] = x.astype(ml_dtypes.bfloat16)

    iota = np.tile(np.arange(128, dtype=np.float32), (128, 1)).astype(ml_dtypes.bfloat16)
    identf = np.eye(128, dtype=np.float32)
    identb = np.eye(128, dtype=ml_dtypes.bfloat16)
    pidx = np.arange(128, dtype=np.float32).reshape(128, 1)

    # weights
    conv_w = np.asarray(inp["conv_w"], dtype=np.float32).reshape(6, F, F)
    convw = conv_w.transpose(1, 0, 2).reshape(F, 6 * F).astype(ml_dtypes.bfloat16)
    convb = np.asarray(inp["conv_b"], dtype=np.float32).reshape(6, F).T.copy()
    jk_w = np.asarray(inp["jk_w"], dtype=np.float32).reshape(NB, 2, F, F).reshape(6, F, F)
    jkw = jk_w.transpose(1, 0, 2).reshape(F, 6 * F).astype(ml_dtypes.bfloat16)
    jkb = np.asarray(inp["jk_b"], dtype=np.float32).T.copy()
    s = (np.asarray(inp["bn_gamma"], dtype=np.float32)
         / np.sqrt(np.asarray(inp["bn_var"], dtype=np.float32) + BN_EPS))
    t = (np.asarray(inp["bn_beta"], dtype=np.float32)
         - np.asarray(inp["bn_mean"], dtype=np.float32) * s)
    bns = s.reshape(NB, F).T.copy()
    bnt = t.reshape(NB, F).T.copy()
    lin1_w = np.asarray(inp["lin1_w"], dtype=np.float32).reshape(NB, F, F)
    l1w = lin1_w.transpose(1, 0, 2).reshape(F, NB * F).copy()
    l1b = np.asarray(inp["lin1_b"], dtype=np.float32).reshape(F, 1).copy()
    l2w = np.asarray(inp["lin2_w"], dtype=np.float32).copy()
    l2b = np.asarray(inp["lin2_b"], dtype=np.float32).reshape(10, 1).copy()

    shared = {
        "iota": iota, "identf": identf, "identb": identb, "pidx": pidx,
        "convw": convw, "convb": convb, "jkw": jkw, "jkb": jkb,
        "bns": bns, "bnt": bnt, "l1w": l1w, "l1b": l1b, "l2w": l2w, "l2b": l2b,
        "xtab": xtab,
    }

    dst_core = core_of[dst]
    dst_off = off[dst]
    src_tab = tab[src]

    in_maps = []
    for c in range(NC):
        eidx = np.flatnonzero(dst_core == c)
        e_win = dst_off[eidx] // 128
        e_k = src_tab[eidx] // CHROWS
        key = e_win * NCH + e_k
        order = np.argsort(key, kind="stable")
        eidx = eidx[order]
        key = key[order]
        counts = np.bincount(key, minlength=NW * NCH)
        assert (counts <= CH * 128).all(), f"chunk overflow core {c}"
        starts = np.concatenate([[0], np.cumsum(counts)])[:-1]
        pos = np.arange(len(eidx)) - starts[key]
        # slot space: [NW, NCH, CH, 128]
        slot = key * (CH * 128) + pos

        idx_slots = np.zeros(NW * NCH * CH * 128, dtype=np.int64)
        rel_slots = np.zeros(NW * NCH * CH * 128, dtype=np.float32)
        nrm_slots = np.zeros(NW * NCH * CH * 128, dtype=np.float32)
        idx_slots[slot] = src_tab[eidx] % CHROWS
        rel_slots[slot] = (dst_off[eidx] % 128).astype(np.float32)
        nrm_slots[slot] = norm[eidx]
        # empty slots: rel stays 0 but norm is 0, so one-hot row is all-zero.

        # gather idx runs, batched WB windows per gather: [NCH, NBAT, NIDX]
        runs = (idx_slots.reshape(NW, NCH, CH * 128)
                .transpose(1, 0, 2).reshape(NCH, NBAT, NIDX))
        gidx = _wrap_idxs(runs.reshape(NCH * NBAT, NIDX))  # [128, NCH*NBAT*ICOLS]
        # one-hot metadata columns: col = w*CPW + k*CH + c
        rel_cols = rel_slots.reshape(NW * NCH * CH, 128).T  # [128, NW*CPW]
        nrm_cols = nrm_slots.reshape(NW * NCH * CH, 128).T
        rel_cols = rel_cols.astype(ml_dtypes.bfloat16).copy()
        nrm_cols = nrm_cols.astype(ml_dtypes.bfloat16).copy()

        # per-node columns
        d2 = np.zeros((128, NW), dtype=np.float32)
        ln = np.arange(sizes[c], dtype=np.int64)
        d2[ln % 128, ln // 128] = dinv2[bounds[c] + ln]
        pool = np.zeros((128, NW * GPC), dtype=ml_dtypes.bfloat16)
        g_of = batch[bounds[c] + ln].astype(np.int64) - c * GPC
        pool[ln % 128, (ln // 128) * GPC + g_of] = 1.0

        x_nm = np.zeros((S, F), dtype=ml_dtypes.bfloat16)
        x_nm[: sizes[c]] = x[bounds[c]: bounds[c + 1]].astype(ml_dtypes.bfloat16)

        m = {"x_nm": x_nm, "gidx": gidx, "rel": rel_cols, "norm": nrm_cols,
             "dinv2": d2, "pool": pool}
        m.update(shared)
        in_maps.append(m)
    return in_maps


def _build_program(stage=99):
    nc = bacc.Bacc("TRN2", target_bir_lowering=False, debug=False,
                   num_devices=NC)
    AF = mybir.ActivationFunctionType
    OP = mybir.AluOpType

    ap = {}
    for name, shape, dt in [
        ("x_nm", [S, F], bf16), ("xtab", [TAB, F], bf16),
        ("gidx", [128, NCH * NBAT * ICOLS], i16),
        ("rel", [128, NW * CPW], bf16), ("norm", [128, NW * CPW], bf16),
        ("dinv2", [128, NW], f32), ("pidx", [128, 1], f32),
        ("pool", [128, NW * GPC], bf16),
        ("iota", [128, 128], bf16), ("identf", [128, 128], f32),
        ("identb", [128, 128], bf16),
        ("convw", [F, 6 * F], bf16), ("convb", [F, 6], f32),
        ("jkw", [F, 6 * F], bf16), ("jkb", [F, NB], f32),
        ("bns", [F, NB], f32), ("bnt", [F, NB], f32),
        ("l1w", [F, NB * F], f32), ("l1b", [F, 1], f32),
        ("l2w", [F, 10], f32), ("l2b", [10, 1], f32),
    ]:
        ap[name] = nc.dram_tensor(name, shape, dt, kind="ExternalInput").ap()
    out_ap = nc.dram_tensor("out", [GPC, 10], f32, kind="ExternalOutput").ap()

    with tile.TileContext(nc) as tc:
        with (
            tc.tile_pool(name="dram", bufs=1, space="DRAM") as dram,
            tc.tile_pool(name="pers", bufs=1) as pers,
            tc.tile_pool(name="rot", bufs=1) as rot,
            tc.tile_pool(name="psum", bufs=1, space="PSUM") as psum,
        ):
            ag_in = dram.tile([S, F], bf16)

            # ---- persistent SBUF loads
            sb = {}
            for name in ["gidx", "rel", "norm", "dinv2", "pidx", "pool",
                         "iota", "identf", "identb", "convw", "convb", "jkw",
                         "jkb", "bns", "bnt", "l1w", "l1b", "l2w", "l2b"]:
                t_ = pers.tile(list(ap[name].shape), ap[name].dtype, name=f"sb_{name}")
                nc.sync.dma_start(t_[:], ap[name][:])
                sb[name] = t_

            h_nm = pers.tile([128, NW, F], bf16, name="h_nm")
            h1_fm = pers.tile([128, S], bf16, name="h1_fm")
            h2_fm = pers.tile([128, S], bf16, name="h2_fm")
            hb_fm = pers.tile([128, S], bf16, name="hb_fm")
            z_sb = pers.tile([128, NB, GPC], f32, name="z_sb")
            agg_sb = pers.tile([128, NW, F], f32, name="agg_sb")
            diag = pers.tile([128, NW, 128], bf16, name="diag")

            # diag(dinv2) blocks, layer-invariant: diag[p, w, d] = (d==p)*dinv2
            for w in range(NW):
                nc.vector.tensor_scalar(
                    out=diag[:, w, :], in0=sb["iota"][:],
                    scalar1=sb["pidx"][:], scalar2=sb["dinv2"][:, w:w + 1],
                    op0=OP.is_equal, op1=OP.mult)

            iota_bc = sb["iota"][:].unsqueeze(1).broadcast_to([128, CH, 128])

            def conv(lk, tables, h_out, write_nm):
                for k in range(NCH):
                    for b in range(NBAT):
                        G = rot.tile([128, WB * CH, F], bf16, tag="G", bufs=3,
                                     name="G")
                        gc = (k * NBAT + b) * ICOLS
                        nc.gpsimd.dma_gather(
                            out_ap=G[:], in_ap=tables[k][:],
                            idxs_ap=sb["gidx"][:, gc:gc + ICOLS],
                            num_idxs=NIDX, num_idxs_reg=NIDX, elem_size=F,
                            single_packet=False)
                        for wi in range(WB):
                            w = b * WB + wi
                            col = w * CPW + k * CH
                            oh = rot.tile([128, CH, 128], bf16, tag="oh",
                                          bufs=6, name="oh")
                            rel_bc = (sb["rel"][:, col:col + CH]
                                      .unsqueeze(2).broadcast_to([128, CH, 128]))
                            nrm_bc = (sb["norm"][:, col:col + CH]
                                      .unsqueeze(2).broadcast_to([128, CH, 128]))
                            nc.vector.tensor_tensor(out=oh[:], in0=iota_bc,
                                                    in1=rel_bc, op=OP.is_equal)
                            nc.vector.tensor_tensor(out=oh[:], in0=oh[:],
                                                    in1=nrm_bc, op=OP.mult)
                            pp = psum.tile([128, F], f32, tag="pp", bufs=4,
                                           name="pp")
                            if k == 0:
                                # self-loop: agg[f,d] += h[d,f]*dinv2[d]
                                nc.tensor.matmul(pp[:], h_nm[:, w, :],
                                                 diag[:, w, :],
                                                 start=True, stop=False)
                            for c in range(CH):
                                nc.tensor.matmul(
                                    pp[:], G[:, wi * CH + c, :], oh[:, c, :],
                                    start=(k != 0 and c == 0),
                                    stop=(c == CH - 1))
                            if k == 0:
                                nc.vector.tensor_copy(agg_sb[:, w, :], pp[:])
                            else:
                                nc.vector.tensor_tensor(
                                    out=agg_sb[:, w, :], in0=agg_sb[:, w, :],
                                    in1=pp[:], op=OP.add)
                                # tail: W matmul + bias + relu (feat-major)
                                tTs = rot.tile([128, F], bf16, tag="tTs",
                                               bufs=3, name="tTs")
                                nc.scalar.copy(tTs[:], agg_sb[:, w, :])
                                hn = psum.tile([128, F], f32, tag="hn", bufs=2,
                                               name="hn")
                                nc.tensor.matmul(
                                    hn[:], sb["convw"][:, lk * F:(lk + 1) * F],
                                    tTs[:], start=True, stop=True)
                                nc.scalar.activation(
                                    h_out[:, w * 128:(w + 1) * 128], hn[:],
                                    AF.Relu, bias=sb["convb"][:, lk:lk + 1])
                                if write_nm:
                                    hnT = psum.tile([128, F], bf16, tag="hnT",
                                                    bufs=1, name="hnT")
                                    nc.tensor.transpose(
                                        hnT[:], h_out[:, w * 128:(w + 1) * 128],
                                        sb["identb"][:])
                                    nc.scalar.copy(h_nm[:, w, :], hnT[:])
                        if k == NCH - 1 and write_nm:
                            r0 = b * WB * 128
                            nc.sync.dma_start(
                                ag_in[r0:r0 + WB * 128, :]
                                .rearrange("(w p) f -> p w f", p=128),
                                h_nm[:, b * WB:(b + 1) * WB, :])

            def allgather(i):
                tabs = []
                for k in range(NCH):
                    tk = dram.tile([CHROWS, F], bf16, addr_space="Shared",
                                   tag=f"t{_rep[0]}_{i}_{k}",
                                   name=f"t{_rep[0]}_{i}_{k}")
                    nc.gpsimd.collective_compute(
                        "AllGather", OP.bypass,
                        replica_groups=[list(range(NC))],
                        ins=[ag_in[k * CHS:(k + 1) * CHS, :].opt()],
                        outs=[tk.opt()])
                    tabs.append(tk)
                return tabs

            def jk(li, last):
                pooled = psum.tile([128, GPC], f32, tag="pooled", bufs=1,
                                   name="pooled")
                for w in range(NW):
                    hb = psum.tile([128, F], f32, tag="hn", bufs=2, name="hb")
                    nc.tensor.matmul(hb[:], sb["jkw"][:, (2 * li) * F:(2 * li + 1) * F],
                                     h1_fm[:, w * 128:(w + 1) * 128],
                                     start=True, stop=False)
                    nc.tensor.matmul(hb[:], sb["jkw"][:, (2 * li + 1) * F:(2 * li + 2) * F],
                                     h2_fm[:, w * 128:(w + 1) * 128],
                                     start=False, stop=True)
                    nc.scalar.activation(hb_fm[:, w * 128:(w + 1) * 128], hb[:],
                                         AF.Relu, bias=sb["jkb"][:, li:li + 1])
                    hnT = psum.tile([128, F], bf16, tag="hnT", bufs=1, name="hnT")
                    nc.tensor.transpose(hnT[:], hb_fm[:, w * 128:(w + 1) * 128],
                                        sb["identb"][:])
                    nc.scalar.copy(h_nm[:, w, :], hnT[:])
                    if not last and (w % WB == WB - 1):
                        r0 = (w - WB + 1) * 128
                        nc.sync.dma_start(
                            ag_in[r0:r0 + WB * 128, :]
                            .rearrange("(w p) f -> p w f", p=128),
                            h_nm[:, w - WB + 1:w + 1, :])
                    nc.tensor.matmul(pooled[:], h_nm[:, w, :],
                                     sb["pool"][:, w * GPC:(w + 1) * GPC],
                                     start=(w == 0), stop=(w == NW - 1))
                nc.scalar.copy(z_sb[:, li, :], pooled[:])

            # ---- main flow
            _rep = [0]
            steps = [
                lambda: conv(0, [ap["xtab"][k * CHROWS:(k + 1) * CHROWS, :] for k in range(NCH)], h1_fm, True),
                lambda: allgather(0),
                lambda t: conv(1, t, h2_fm, False),
                lambda: jk(0, False),
                lambda: allgather(1),
                lambda t: conv(2, t, h1_fm, True),
                lambda: allgather(2),
                lambda t: conv(3, t, h2_fm, False),
                lambda: jk(1, False),
                lambda: allgather(3),
                lambda t: conv(4, t, h1_fm, True),
                lambda: allgather(4),
                lambda t: conv(5, t, h2_fm, False),
                lambda: jk(2, True),
            ]
            for rep in range(REPEAT):
                _rep[0] = rep
                for b in range(NBAT):
                    nc.sync.dma_start(
                        h_nm[:, b * WB:(b + 1) * WB, :],
                        ap["x_nm"][b * WB * 128:(b + 1) * WB * 128, :]
                        .rearrange("(w p) f -> p w f", p=128))
                table = None
                for i, st in enumerate(steps):
                    if i >= stage:
                        break
                    r = st(table) if st.__code__.co_argcount else st()
                    if r is not None:
                        table = r

            # ---- head
            if stage < 14:
                outt0 = rot.tile([GPC, 10], f32, tag="outt", bufs=1, name="outt0")
                nc.vector.tensor_copy(outt0[:], h1_fm[0:GPC, 0:10])
                nc.sync.dma_start(out_ap[:], outt0[:])
            else:
                _head(nc, tc, rot, psum, sb, z_sb, out_ap)

    nc.compile()
    return nc


def _head(nc, tc, rot, psum, sb, z_sb, out_ap):
    AF = mybir.ActivationFunctionType
    OP = mybir.AluOpType
    zbn = rot.tile([128, NB, GPC], f32, tag="zbn", bufs=1, name="zbn")
    for t in range(NB):
        nc.vector.tensor_scalar(
            out=zbn[:, t, :], in0=z_sb[:, t, :],
            scalar1=sb["bns"][:, t:t + 1], scalar2=sb["bnt"][:, t:t + 1],
            op0=OP.mult, op1=OP.add)
    a1 = psum.tile([128, GPC], f32, tag="hn", bufs=2, name="a1")
    for t in range(NB):
        nc.tensor.matmul(a1[:], sb["l1w"][:, t * F:(t + 1) * F],
                         zbn[:, t, :], start=(t == 0), stop=(t == NB - 1))
    a1s = rot.tile([128, GPC], f32, tag="a1s", bufs=1, name="a1s")
    nc.scalar.activation(a1s[:], a1[:], AF.Relu, bias=sb["l1b"][:])
    z2 = psum.tile([10, GPC], f32, tag="pooled", bufs=1, name="z2")
    nc.tensor.matmul(z2[:], sb["l2w"][:], a1s[:], start=True, stop=True)
    z2s = rot.tile([10, GPC], f32, tag="z2s", bufs=1, name="z2s")
    nc.scalar.activation(z2s[:], z2[:], AF.Identity, bias=sb["l2b"][:])
    z2T = psum.tile([GPC, 10], f32, tag="hnT", bufs=1, name="z2T")
    nc.tensor.transpose(z2T[:], z2s[:], sb["identf"][0:10, 0:10])
    z2Ts = rot.tile([GPC, 10], f32, tag="z2Ts", bufs=1, name="z2Ts")
    nc.vector.tensor_copy(z2Ts[:], z2T[:])
    negm = rot.tile([GPC, 1], f32, tag="negm", bufs=1, name="negm")
    nc.vector.tensor_reduce(negm[:], z2Ts[:], mybir.AxisListType.X,
                            OP.max, negate=True)
    et = rot.tile([GPC, 10], f32, tag="et", bufs=1, name="et")
    nc.scalar.activation(et[:], z2Ts[:], AF.Exp, bias=negm[:])
    ssum = rot.tile([GPC, 1], f32, tag="ssum", bufs=1, name="ssum")
    nc.vector.tensor_reduce(ssum[:], et[:], mybir.AxisListType.X, OP.add)
    rcp = rot.tile([GPC, 1], f32, tag="rcp", bufs=1, name="rcp")
    nc.vector.reciprocal(rcp[:], ssum[:])
    outt = rot.tile([GPC, 10], f32, tag="outt", bufs=1, name="outt")
    nc.vector.tensor_scalar_mul(outt[:], et[:], rcp[:])
    nc.sync.dma_start(out_ap[:], outt[:])


def _get_program():
    global _PROGRAM
    if _PROGRAM is None:
        _PROGRAM = _build_program()
    return _PROGRAM


def kernel(**inputs) -> np.ndarray:
    in_maps = _preprocess(inputs)
    nc = _get_program()
    res = run_bass_kernel_spmd(nc, in_maps, list(range(NC)))
    return np.concatenate([res.results[c]["out"] for c in range(NC)], axis=0)


# revision 11
# speedup vs baseline: 1.1269x; 1.1269x over previous
"""Trainium2 Bass kernel for nn_BaseModel_14499809591724 (GNN message passing).

Strategy (8 NeuronCores, data-parallel over graph batches):
  - Nodes are split into 8 contiguous shards at graph boundaries (batch is
    sorted), padded to S=6400 rows each; full node table = [8*S, 128] bf16.
  - Each core owns the edges whose dst falls in its shard. Edges are sorted by
    (dst window of 128 nodes, src-table chunk) and chunked into groups of 128.
  - Per GCN conv: batched dma_gather of h[src] rows (WB windows per call) from
    the replicated DRAM table; scaled one-hots for a whole (window, chunk) are
    built with TWO wide DVE tensor_tensor ops using broadcast (stride-0) APs;
    scatter-reduce via PE matmul with the GATHERED rows stationary, producing
    feat-major agg directly (no transpose); self-loop term is one extra matmul
    against a precomputed diag(dinv2) block; then W + bias + ReLU.
  - After each conv that feeds another conv, the 8 local shards are AllGathered
    (bf16, 2 chunks for progressive overlap) to rebuild the replicated table.
  - JumpingKnowledge + per-graph pooling (one-hot matmul) + BN + MLP head +
    softmax run per core on its own 64 graphs; host concatenates 8 x [64, 10].

All heavy compute runs on device. Host does index/layout preprocessing and
edge-weight normalization (deg/dinv/norm), which is sharding metadata.
"""
import sys
import numpy as np
import ml_dtypes

sys.path.insert(0, "/opt/trn_rl_repo")

from concourse import bacc, tile, mybir  # noqa: E402
from concourse.bass_utils import run_bass_kernel_spmd  # noqa: E402

# ---- model / sharding constants (shapes fixed by the problem) ----
NC = 8
N_NODES = 50000
N_EDGES = 800000
F = 128
B = 512
GPC = B // NC          # graphs per core = 64
S = 6400               # padded nodes per shard (max real shard is 6368)
NW = S // 128          # 50 windows per core
TAB = NC * S           # 51200 table rows
NCH = 2                # table chunks (progressive AllGather pipeline)
CHS = S // NCH         # 3200 shard rows per chunk
CHROWS = NC * CHS      # 25600 table rows per chunk (int16-safe)
CH = 9                 # 128-edge groups per (window, table-chunk); max seen 1112
CPW = NCH * CH         # 18 one-hot columns per window
WB = 5                 # windows per dma_gather batch
NBAT = NW // WB        # 10 gather batches per (conv, chunk)
NIDX = WB * CH * 128   # 5760 idxs per gather
ICOLS = NIDX // 16     # 360 wrapped idx columns per gather
NB = 3
BN_EPS = 1e-5

f32 = mybir.dt.float32
bf16 = mybir.dt.bfloat16
i16 = mybir.dt.int16

_PROGRAM = None
import os
REPEAT = int(os.environ.get("REPEAT", "1"))


def _wrap_idxs(runs: np.ndarray) -> np.ndarray:
    """[R, NIDX] int -> [128, R*ICOLS] int16 (16-partition wrap, 8x replicated)."""
    r = runs.shape[0]
    w = runs.reshape(r, -1, 16).transpose(2, 0, 1).reshape(16, -1)
    return np.tile(w.astype(np.int16), (8, 1))


def _preprocess(inp: dict):
    batch = np.asarray(inp["batch"])
    ei = np.asarray(inp["edge_index"])
    ew = np.asarray(inp["edge_attr"], dtype=np.float32)
    x = np.asarray(inp["x"], dtype=np.float32)
    src, dst = ei[0].astype(np.int64), ei[1].astype(np.int64)

    bounds = np.searchsorted(batch, np.arange(0, B + 1, GPC)).astype(np.int64)
    sizes = np.diff(bounds)
    assert sizes.max() <= S, f"shard overflow: {sizes.max()} > {S}"

    node = np.arange(N_NODES, dtype=np.int64)
    core_of = (np.searchsorted(bounds, node, side="right") - 1).astype(np.int64)
    off = node - bounds[core_of]
    # chunk-major table: row = chunk*CHROWS + core*CHS + (off % CHS)
    tab = (off // CHS) * CHROWS + core_of * CHS + (off % CHS)

    deg = (np.bincount(dst, weights=ew.astype(np.float64), minlength=N_NODES) + 1.0)
    deg = deg.astype(np.float32)
    dinv = 1.0 / np.sqrt(deg)
    norm = (dinv[src] * ew * dinv[dst]).astype(np.float32)
    dinv2 = (1.0 / deg).astype(np.float32)

    # full replicated x table (node-major, bf16)
    xtab = np.zeros((TAB, F), dtype=ml_dtypes.bfloat16)
    xtab[tab] = x.astype(ml_dtypes.bfloat16)

    iota = np.tile(np.arange(128, dtype=np.float32), (128, 1)).astype(ml_dtypes.bfloat16)
    identf = np.eye(128, dtype=np.float32)
    identb = np.eye(128, dtype=ml_dtypes.bfloat16)
    pidx = np.arange(128, dtype=np.float32).reshape(128, 1)

    # weights
    conv_w = np.asarray(inp["conv_w"], dtype=np.float32).reshape(6, F, F)
    convw = conv_w.transpose(1, 0, 2).reshape(F, 6 * F).astype(ml_dtypes.bfloat16)
    convb = np.asarray(inp["conv_b"], dtype=np.float32).reshape(6, F).T.copy()
    jk_w = np.asarray(inp["jk_w"], dtype=np.float32).reshape(NB, 2, F, F).reshape(6, F, F)
    jkw = jk_w.transpose(1, 0, 2).reshape(F, 6 * F).astype(ml_dtypes.bfloat16)
    jkb = np.asarray(inp["jk_b"], dtype=np.float32).T.copy()
    s = (np.asarray(inp["bn_gamma"], dtype=np.float32)
         / np.sqrt(np.asarray(inp["bn_var"], dtype=np.float32) + BN_EPS))
    t = (np.asarray(inp["bn_beta"], dtype=np.float32)
         - np.asarray(inp["bn_mean"], dtype=np.float32) * s)
    bns = s.reshape(NB, F).T.copy()
    bnt = t.reshape(NB, F).T.copy()
    lin1_w = np.asarray(inp["lin1_w"], dtype=np.float32).reshape(NB, F, F)
    l1w = lin1_w.transpose(1, 0, 2).reshape(F, NB * F).copy()
    l1b = np.asarray(inp["lin1_b"], dtype=np.float32).reshape(F, 1).copy()
    l2w = np.asarray(inp["lin2_w"], dtype=np.float32).copy()
    l2b = np.asarray(inp["lin2_b"], dtype=np.float32).reshape(10, 1).copy()

    shared = {
        "iota": iota, "identf": identf, "identb": identb, "pidx": pidx,
        "convw": convw, "convb": convb, "jkw": jkw, "jkb": jkb,
        "bns": bns, "bnt": bnt, "l1w": l1w, "l1b": l1b, "l2w": l2w, "l2b": l2b,
        "xtab": xtab,
    }

    dst_core = core_of[dst]
    dst_off = off[dst]
    src_tab = tab[src]

    in_maps = []
    for c in range(NC):
        eidx = np.flatnonzero(dst_core == c)
        e_win = dst_off[eidx] // 128
        e_k = src_tab[eidx] // CHROWS
        key = e_win * NCH + e_k
        order = np.argsort(key, kind="stable")
        eidx = eidx[order]
        key = key[order]
        counts = np.bincount(key, minlength=NW * NCH)
        assert (counts <= CH * 128).all(), f"chunk overflow core {c}"
        starts = np.concatenate([[0], np.cumsum(counts)])[:-1]
        pos = np.arange(len(eidx)) - starts[key]
        # slot space: [NW, NCH, CH, 128]
        slot = key * (CH * 128) + pos

        idx_slots = np.zeros(NW * NCH * CH * 128, dtype=np.int64)
        rel_slots = np.zeros(NW * NCH * CH * 128, dtype=np.float32)
        nrm_slots = np.zeros(NW * NCH * CH * 128, dtype=np.float32)
        idx_slots[slot] = src_tab[eidx] % CHROWS
        rel_slots[slot] = (dst_off[eidx] % 128).astype(np.float32)
        nrm_slots[slot] = norm[eidx]
        # empty slots: rel stays 0 but norm is 0, so one-hot row is all-zero.

        # gather idx runs, batched WB windows per gather: [NCH, NBAT, NIDX]
        runs = (idx_slots.reshape(NW, NCH, CH * 128)
                .transpose(1, 0, 2).reshape(NCH, NBAT, NIDX))
        gidx = _wrap_idxs(runs.reshape(NCH * NBAT, NIDX))  # [128, NCH*NBAT*ICOLS]
        # precomputed scaled one-hots (layer-invariant): [128, NW*CPW, 128]
        rel_i = rel_slots.reshape(NW * NCH * CH, 128).T.astype(np.int64)
        nrm_c = nrm_slots.reshape(NW * NCH * CH, 128).T
        ohtab = np.zeros((128, NW * CPW, 128), dtype=ml_dtypes.bfloat16)
        np.put_along_axis(ohtab, rel_i[:, :, None],
                          nrm_c[:, :, None].astype(ml_dtypes.bfloat16), axis=2)

        # per-node columns
        d2 = np.zeros((128, NW), dtype=np.float32)
        ln = np.arange(sizes[c], dtype=np.int64)
        d2[ln % 128, ln // 128] = dinv2[bounds[c] + ln]
        pool = np.zeros((128, NW * GPC), dtype=ml_dtypes.bfloat16)
        g_of = batch[bounds[c] + ln].astype(np.int64) - c * GPC
        pool[ln % 128, (ln // 128) * GPC + g_of] = 1.0

        x_nm = np.zeros((S, F), dtype=ml_dtypes.bfloat16)
        x_nm[: sizes[c]] = x[bounds[c]: bounds[c + 1]].astype(ml_dtypes.bfloat16)

        m = {"x_nm": x_nm, "gidx": gidx, "ohtab": ohtab.reshape(128, -1),
             "dinv2": d2, "pool": pool}
        m.update(shared)
        in_maps.append(m)
    return in_maps


def _build_program(stage=99):
    nc = bacc.Bacc("TRN2", target_bir_lowering=False, debug=False,
                   num_devices=NC)
    AF = mybir.ActivationFunctionType
    OP = mybir.AluOpType

    ap = {}
    for name, shape, dt in [
        ("x_nm", [S, F], bf16), ("xtab", [TAB, F], bf16),
        ("gidx", [128, NCH * NBAT * ICOLS], i16),
        ("ohtab", [128, NW * CPW * 128], bf16),
        ("dinv2", [128, NW], f32), ("pidx", [128, 1], f32),
        ("pool", [128, NW * GPC], bf16),
        ("iota", [128, 128], bf16), ("identf", [128, 128], f32),
        ("identb", [128, 128], bf16),
        ("convw", [F, 6 * F], bf16), ("convb", [F, 6], f32),
        ("jkw", [F, 6 * F], bf16), ("jkb", [F, NB], f32),
        ("bns", [F, NB], f32), ("bnt", [F, NB], f32),
        ("l1w", [F, NB * F], f32), ("l1b", [F, 1], f32),
        ("l2w", [F, 10], f32), ("l2b", [10, 1], f32),
    ]:
        ap[name] = nc.dram_tensor(name, shape, dt, kind="ExternalInput").ap()
    out_ap = nc.dram_tensor("out", [GPC, 10], f32, kind="ExternalOutput").ap()

    with tile.TileContext(nc) as tc:
        with (
            tc.tile_pool(name="dram", bufs=1, space="DRAM") as dram,
            tc.tile_pool(name="pers", bufs=1) as pers,
            tc.tile_pool(name="rot", bufs=1) as rot,
            tc.tile_pool(name="psum", bufs=1, space="PSUM") as psum,
        ):
            ag_in = dram.tile([S, F], bf16)

            # ---- persistent SBUF loads
            sb = {}
            for name in ["gidx", "dinv2", "pidx", "pool",
                         "iota", "identf", "identb", "convw", "convb", "jkw",
                         "jkb", "bns", "bnt", "l1w", "l1b", "l2w", "l2b"]:
                t_ = pers.tile(list(ap[name].shape), ap[name].dtype, name=f"sb_{name}")
                nc.sync.dma_start(t_[:], ap[name][:])
                sb[name] = t_

            h_nm = pers.tile([128, NW, F], bf16, name="h_nm")
            h1_fm = pers.tile([128, S], bf16, name="h1_fm")
            h2_fm = pers.tile([128, S], bf16, name="h2_fm")
            hb_fm = pers.tile([128, S], bf16, name="hb_fm")
            z_sb = pers.tile([128, NB, GPC], f32, name="z_sb")
            agg_sb = pers.tile([128, NW, F], f32, name="agg_sb")
            diag = pers.tile([128, NW, 128], bf16, name="diag")

            # diag(dinv2) blocks, layer-invariant: diag[p, w, d] = (d==p)*dinv2
            for w in range(NW):
                nc.vector.tensor_scalar(
                    out=diag[:, w, :], in0=sb["iota"][:],
                    scalar1=sb["pidx"][:], scalar2=sb["dinv2"][:, w:w + 1],
                    op0=OP.is_equal, op1=OP.mult)

            # ohtab viewed as [128, NW, CPW*128]: per-(k,b) slice is
            # [128, WB windows, CH*128] with window stride CPW*128.
            oh3 = ap["ohtab"].rearrange("p (w x) -> p w x", w=NW)

            def conv(lk, tables, h_out, write_nm):
                for k in range(NCH):
                    for b in range(NBAT):
                        G = rot.tile([128, WB * CH, F], bf16, tag="G", bufs=2,
                                     name="G")
                        gc = (k * NBAT + b) * ICOLS
                        nc.gpsimd.dma_gather(
                            out_ap=G[:], in_ap=tables[k][:],
                            idxs_ap=sb["gidx"][:, gc:gc + ICOLS],
                            num_idxs=NIDX, num_idxs_reg=NIDX, elem_size=F,
                            single_packet=False)
                        oh = rot.tile([128, WB, CH, 128], bf16, tag="oh",
                                      bufs=2, name="oh")
                        nc.sync.dma_start(
                            oh[:],
                            oh3[:, b * WB:(b + 1) * WB,
                                k * CH * 128:(k + 1) * CH * 128]
                            .rearrange("p w (c d) -> p w c d", c=CH))
                        for wi in range(WB):
                            w = b * WB + wi
                            pp = psum.tile([128, F], f32, tag="pp", bufs=4,
                                           name="pp")
                            if k == 0:
                                # self-loop: agg[f,d] += h[d,f]*dinv2[d]
                                nc.tensor.matmul(pp[:], h_nm[:, w, :],
                                                 diag[:, w, :],
                                                 start=True, stop=False)
                            for c in range(CH):
                                nc.tensor.matmul(
                                    pp[:], G[:, wi * CH + c, :], oh[:, wi, c, :],
                                    start=(k != 0 and c == 0),
                                    stop=(c == CH - 1))
                            if k == 0:
                                nc.vector.tensor_copy(agg_sb[:, w, :], pp[:])
                            else:
                                nc.vector.tensor_tensor(
                                    out=agg_sb[:, w, :], in0=agg_sb[:, w, :],
                                    in1=pp[:], op=OP.add)
                                # tail: W matmul + bias + relu (feat-major)
                                tTs = rot.tile([128, F], bf16, tag="tTs",
                                               bufs=3, name="tTs")
                                nc.scalar.copy(tTs[:], agg_sb[:, w, :])
                                hn = psum.tile([128, F], f32, tag="hn", bufs=2,
                                               name="hn")
                                nc.tensor.matmul(
                                    hn[:], sb["convw"][:, lk * F:(lk + 1) * F],
                                    tTs[:], start=True, stop=True)
                                nc.scalar.activation(
                                    h_out[:, w * 128:(w + 1) * 128], hn[:],
                                    AF.Relu, bias=sb["convb"][:, lk:lk + 1])
                                if write_nm:
                                    hnT = psum.tile([128, F], bf16, tag="hnT",
                                                    bufs=1, name="hnT")
                                    nc.tensor.transpose(
                                        hnT[:], h_out[:, w * 128:(w + 1) * 128],
                                        sb["identb"][:])
                                    nc.scalar.copy(h_nm[:, w, :], hnT[:])
                        if k == NCH - 1 and write_nm:
                            r0 = b * WB * 128
                            nc.sync.dma_start(
                                ag_in[r0:r0 + WB * 128, :]
                                .rearrange("(w p) f -> p w f", p=128),
                                h_nm[:, b * WB:(b + 1) * WB, :])

            def allgather(i):
                tabs = []
                for k in range(NCH):
                    tk = dram.tile([CHROWS, F], bf16, addr_space="Shared",
                                   tag=f"t{_rep[0]}_{i}_{k}",
                                   name=f"t{_rep[0]}_{i}_{k}")
                    nc.gpsimd.collective_compute(
                        "AllGather", OP.bypass,
                        replica_groups=[list(range(NC))],
                        ins=[ag_in[k * CHS:(k + 1) * CHS, :].opt()],
                        outs=[tk.opt()])
                    tabs.append(tk)
                return tabs

            def jk(li, last):
                pooled = psum.tile([128, GPC], f32, tag="pooled", bufs=1,
                                   name="pooled")
                for w in range(NW):
                    hb = psum.tile([128, F], f32, tag="hn", bufs=2, name="hb")
                    nc.tensor.matmul(hb[:], sb["jkw"][:, (2 * li) * F:(2 * li + 1) * F],
                                     h1_fm[:, w * 128:(w + 1) * 128],
                                     start=True, stop=False)
                    nc.tensor.matmul(hb[:], sb["jkw"][:, (2 * li + 1) * F:(2 * li + 2) * F],
                                     h2_fm[:, w * 128:(w + 1) * 128],
                                     start=False, stop=True)
                    nc.scalar.activation(hb_fm[:, w * 128:(w + 1) * 128], hb[:],
                                         AF.Relu, bias=sb["jkb"][:, li:li + 1])
                    hnT = psum.tile([128, F], bf16, tag="hnT", bufs=1, name="hnT")
                    nc.tensor.transpose(hnT[:], hb_fm[:, w * 128:(w + 1) * 128],
                                        sb["identb"][:])
                    nc.scalar.copy(h_nm[:, w, :], hnT[:])
                    if not last and (w % WB == WB - 1):
                        r0 = (w - WB + 1) * 128
                        nc.sync.dma_start(
                            ag_in[r0:r0 + WB * 128, :]
                            .rearrange("(w p) f -> p w f", p=128),
                            h_nm[:, w - WB + 1:w + 1, :])
                    nc.tensor.matmul(pooled[:], h_nm[:, w, :],
                                     sb["pool"][:, w * GPC:(w + 1) * GPC],
                                     start=(w == 0), stop=(w == NW - 1))
                nc.scalar.copy(z_sb[:, li, :], pooled[:])

            # ---- main flow
            _rep = [0]
            steps = [
                lambda: conv(0, [ap["xtab"][k * CHROWS:(k + 1) * CHROWS, :] for k in range(NCH)], h1_fm, True),
                lambda: allgather(0),
                lambda t: conv(1, t, h2_fm, False),
                lambda: jk(0, False),
                lambda: allgather(1),
                lambda t: conv(2, t, h1_fm, True),
                lambda: allgather(2),
                lambda t: conv(3, t, h2_fm, False),
                lambda: jk(1, False),
                lambda: allgather(3),
                lambda t: conv(4, t, h1_fm, True),
                lambda: allgather(4),
                lambda t: conv(5, t, h2_fm, False),
                lambda: jk(2, True),
            ]
            for rep in range(REPEAT):
                _rep[0] = rep
                for b in range(NBAT):
                    nc.sync.dma_start(
                        h_nm[:, b * WB:(b + 1) * WB, :],
                        ap["x_nm"][b * WB * 128:(b + 1) * WB * 128, :]
                        .rearrange("(w p) f -> p w f", p=128))
                table = None
                for i, st in enumerate(steps):
                    if i >= stage:
                        break
                    r = st(table) if st.__code__.co_argcount else st()
                    if r is not None:
                        table = r

            # ---- head
            if stage < 14:
                outt0 = rot.tile([GPC, 10], f32, tag="outt", bufs=1, name="outt0")
                nc.vector.tensor_copy(outt0[:], h1_fm[0:GPC, 0:10])
                nc.sync.dma_start(out_ap[:], outt0[:])
            else:
                _head(nc, tc, rot, psum, sb, z_sb, out_ap)

    nc.compile()
    return nc


def _head(nc, tc, rot, psum, sb, z_sb, out_ap):
    AF = mybir.ActivationFunctionType
    OP = mybir.AluOpType
    zbn = rot.tile([128, NB, GPC], f32, tag="zbn", bufs=1, name="zbn")
    for t in range(NB):
        nc.vector.tensor_scalar(
            out=zbn[:, t, :], in0=z_sb[:, t, :],
            scalar1=sb["bns"][:, t:t + 1], scalar2=sb["bnt"][:, t:t + 1],
            op0=OP.mult, op1=OP.add)
    a1 = psum.tile([128, GPC], f32, tag="hn", bufs=2, name="a1")
    for t in range(NB):
        nc.tensor.matmul(a1[:], sb["l1w"][:, t * F:(t + 1) * F],
                         zbn[:, t, :], start=(t == 0), stop=(t == NB - 1))
    a1s = rot.tile([128, GPC], f32, tag="a1s", bufs=1, name="a1s")
    nc.scalar.activation(a1s[:], a1[:], AF.Relu, bias=sb["l1b"][:])
    z2 = psum.tile([10, GPC], f32, tag="pooled", bufs=1, name="z2")
    nc.tensor.matmul(z2[:], sb["l2w"][:], a1s[:], start=True, stop=True)
    z2s = rot.tile([10, GPC], f32, tag="z2s", bufs=1, name="z2s")
    nc.scalar.activation(z2s[:], z2[:], AF.Identity, bias=sb["l2b"][:])
    z2T = psum.tile([GPC, 10], f32, tag="hnT", bufs=1, name="z2T")
    nc.tensor.transpose(z2T[:], z2s[:], sb["identf"][0:10, 0:10])
    z2Ts = rot.tile([GPC, 10], f32, tag="z2Ts", bufs=1, name="z2Ts")
    nc.vector.tensor_copy(z2Ts[:], z2T[:])
    negm = rot.tile([GPC, 1], f32, tag="negm", bufs=1, name="negm")
    nc.vector.tensor_reduce(negm[:], z2Ts[:], mybir.AxisListType.X,
                            OP.max, negate=True)
    et = rot.tile([GPC, 10], f32, tag="et", bufs=1, name="et")
    nc.scalar.activation(et[:], z2Ts[:], AF.Exp, bias=negm[:])
    ssum = rot.tile([GPC, 1], f32, tag="ssum", bufs=1, name="ssum")
    nc.vector.tensor_reduce(ssum[:], et[:], mybir.AxisListType.X, OP.add)
    rcp = rot.tile([GPC, 1], f32, tag="rcp", bufs=1, name="rcp")
    nc.vector.reciprocal(rcp[:], ssum[:])
    outt = rot.tile([GPC, 10], f32, tag="outt", bufs=1, name="outt")
    nc.vector.tensor_scalar_mul(outt[:], et[:], rcp[:])
    nc.sync.dma_start(out_ap[:], outt[:])


def _get_program():
    global _PROGRAM
    if _PROGRAM is None:
        _PROGRAM = _build_program()
    return _PROGRAM


def kernel(**inputs) -> np.ndarray:
    in_maps = _preprocess(inputs)
    nc = _get_program()
    res = run_bass_kernel_spmd(nc, in_maps, list(range(NC)))
    return np.concatenate([res.results[c]["out"] for c in range(NC)], axis=0)


# revision 13
# speedup vs baseline: 1.4793x; 1.3127x over previous
"""Trainium2 Bass kernel for nn_BaseModel_14499809591724 (GNN message passing).

Strategy (8 NeuronCores, data-parallel over graph batches):
  - Nodes are split into 8 contiguous shards at graph boundaries (batch is
    sorted), padded to S=6400 rows each; full node table = [8*S, 128] bf16.
  - Each core owns the edges whose dst falls in its shard. Edges are sorted by
    (dst window of 128 nodes, src-table chunk) and chunked into groups of 128.
  - Per GCN conv: batched dma_gather of h[src] rows (WB windows per call) from
    the replicated DRAM table; scaled one-hots for a whole (window, chunk) are
    built with TWO wide DVE tensor_tensor ops using broadcast (stride-0) APs;
    scatter-reduce via PE matmul with the GATHERED rows stationary, producing
    feat-major agg directly (no transpose); self-loop term is one extra matmul
    against a precomputed diag(dinv2) block; then W + bias + ReLU.
  - After each conv that feeds another conv, the 8 local shards are AllGathered
    (bf16, 2 chunks for progressive overlap) to rebuild the replicated table.
  - JumpingKnowledge + per-graph pooling (one-hot matmul) + BN + MLP head +
    softmax run per core on its own 64 graphs; host concatenates 8 x [64, 10].

All heavy compute runs on device. Host does index/layout preprocessing and
edge-weight normalization (deg/dinv/norm), which is sharding metadata.
"""
import sys
import numpy as np
import ml_dtypes

sys.path.insert(0, "/opt/trn_rl_repo")

from concourse import bacc, tile, mybir  # noqa: E402
from concourse.bass_utils import run_bass_kernel_spmd  # noqa: E402

# ---- model / sharding constants (shapes fixed by the problem) ----
NC = 8
N_NODES = 50000
N_EDGES = 800000
F = 128
B = 512
GPC = B // NC          # graphs per core = 64
S = 6400               # padded nodes per shard (max real shard is 6368)
NW = S // 128          # 50 windows per core
TAB = NC * S           # 51200 table rows
NCH = 2                # table chunks (progressive AllGather pipeline)
CHS = S // NCH         # 3200 shard rows per chunk
CHROWS = NC * CHS      # 25600 table rows per chunk (int16-safe)
CH = 9                 # 128-edge groups per (window, table-chunk); max seen 1112
CPW = NCH * CH         # 18 one-hot columns per window
WB = 5                 # windows per dma_gather batch
NBAT = NW // WB        # 10 gather batches per (conv, chunk)
NIDX = WB * CH * 128   # 5760 idxs per gather
ICOLS = NIDX // 16     # 360 wrapped idx columns per gather
NB = 3
BN_EPS = 1e-5

f32 = mybir.dt.float32
bf16 = mybir.dt.bfloat16
i16 = mybir.dt.int16

_PROGRAM = None
import os
REPEAT = int(os.environ.get("REPEAT", "1"))


def _wrap_idxs(runs: np.ndarray) -> np.ndarray:
    """[R, NIDX] int -> [128, R*ICOLS] int16 (16-partition wrap, 8x replicated)."""
    r = runs.shape[0]
    w = runs.reshape(r, -1, 16).transpose(2, 0, 1).reshape(16, -1)
    return np.tile(w.astype(np.int16), (8, 1))


def _preprocess(inp: dict):
    batch = np.asarray(inp["batch"])
    ei = np.asarray(inp["edge_index"])
    ew = np.asarray(inp["edge_attr"], dtype=np.float32)
    x = np.asarray(inp["x"], dtype=np.float32)
    src, dst = ei[0].astype(np.int64), ei[1].astype(np.int64)

    bounds = np.searchsorted(batch, np.arange(0, B + 1, GPC)).astype(np.int64)
    sizes = np.diff(bounds)
    assert sizes.max() <= S, f"shard overflow: {sizes.max()} > {S}"

    node = np.arange(N_NODES, dtype=np.int64)
    core_of = (np.searchsorted(bounds, node, side="right") - 1).astype(np.int64)
    off = node - bounds[core_of]
    # chunk-major table: row = chunk*CHROWS + core*CHS + (off % CHS)
    tab = (off // CHS) * CHROWS + core_of * CHS + (off % CHS)

    deg = (np.bincount(dst, weights=ew.astype(np.float64), minlength=N_NODES) + 1.0)
    deg = deg.astype(np.float32)
    dinv = 1.0 / np.sqrt(deg)
    norm = (dinv[src] * ew * dinv[dst]).astype(np.float32)
    dinv2 = (1.0 / deg).astype(np.float32)

    # full replicated x table (node-major, bf16)
    xtab = np.zeros((TAB, F), dtype=ml_dtypes.bfloat16)
    xtab[tab] = x.astype(ml_dtypes.bfloat16)

    iota = np.tile(np.arange(128, dtype=np.float32), (128, 1)).astype(ml_dtypes.bfloat16)
    identf = np.eye(128, dtype=np.float32)
    identb = np.eye(128, dtype=ml_dtypes.bfloat16)
    pidx = np.arange(128, dtype=np.float32).reshape(128, 1)

    # weights
    conv_w = np.asarray(inp["conv_w"], dtype=np.float32).reshape(6, F, F)
    convw = conv_w.transpose(1, 0, 2).reshape(F, 6 * F).astype(ml_dtypes.bfloat16)
    convb = np.asarray(inp["conv_b"], dtype=np.float32).reshape(6, F).T.copy()
    jk_w = np.asarray(inp["jk_w"], dtype=np.float32).reshape(NB, 2, F, F).reshape(6, F, F)
    jkw = jk_w.transpose(1, 0, 2).reshape(F, 6 * F).astype(ml_dtypes.bfloat16)
    jkb = np.asarray(inp["jk_b"], dtype=np.float32).T.copy()
    s = (np.asarray(inp["bn_gamma"], dtype=np.float32)
         / np.sqrt(np.asarray(inp["bn_var"], dtype=np.float32) + BN_EPS))
    t = (np.asarray(inp["bn_beta"], dtype=np.float32)
         - np.asarray(inp["bn_mean"], dtype=np.float32) * s)
    bns = s.reshape(NB, F).T.copy()
    bnt = t.reshape(NB, F).T.copy()
    lin1_w = np.asarray(inp["lin1_w"], dtype=np.float32).reshape(NB, F, F)
    l1w = lin1_w.transpose(1, 0, 2).reshape(F, NB * F).copy()
    l1b = np.asarray(inp["lin1_b"], dtype=np.float32).reshape(F, 1).copy()
    l2w = np.asarray(inp["lin2_w"], dtype=np.float32).copy()
    l2b = np.asarray(inp["lin2_b"], dtype=np.float32).reshape(10, 1).copy()

    shared = {
        "iota": iota, "identf": identf, "identb": identb, "pidx": pidx,
        "convw": convw, "convb": convb, "jkw": jkw, "jkb": jkb,
        "bns": bns, "bnt": bnt, "l1w": l1w, "l1b": l1b, "l2w": l2w, "l2b": l2b,
        "xtab": xtab,
    }

    dst_core = core_of[dst]
    dst_off = off[dst]
    src_tab = tab[src]

    in_maps = []
    for c in range(NC):
        eidx = np.flatnonzero(dst_core == c)
        e_win = dst_off[eidx] // 128
        e_k = src_tab[eidx] // CHROWS
        key = e_win * NCH + e_k
        order = np.argsort(key, kind="stable")
        eidx = eidx[order]
        key = key[order]
        counts = np.bincount(key, minlength=NW * NCH)
        assert (counts <= CH * 128).all(), f"chunk overflow core {c}"
        starts = np.concatenate([[0], np.cumsum(counts)])[:-1]
        pos = np.arange(len(eidx)) - starts[key]
        # slot space: [NW, NCH, CH, 128]
        slot = key * (CH * 128) + pos

        idx_slots = np.zeros(NW * NCH * CH * 128, dtype=np.int64)
        rel_slots = np.zeros(NW * NCH * CH * 128, dtype=np.float32)
        nrm_slots = np.zeros(NW * NCH * CH * 128, dtype=np.float32)
        idx_slots[slot] = src_tab[eidx] % CHROWS
        rel_slots[slot] = (dst_off[eidx] % 128).astype(np.float32)
        nrm_slots[slot] = norm[eidx]
        # empty slots: rel stays 0 but norm is 0, so one-hot row is all-zero.

        # gather idx runs, batched WB windows per gather: [NCH, NBAT, NIDX]
        runs = (idx_slots.reshape(NW, NCH, CH * 128)
                .transpose(1, 0, 2).reshape(NCH, NBAT, NIDX))
        gidx = _wrap_idxs(runs.reshape(NCH * NBAT, NIDX))  # [128, NCH*NBAT*ICOLS]
        # precomputed scaled one-hots (layer-invariant): [128, NW*CPW, 128]
        rel_i = rel_slots.reshape(NW * NCH * CH, 128).T.astype(np.int64)
        nrm_c = nrm_slots.reshape(NW * NCH * CH, 128).T
        ohtab = np.zeros((128, NW * CPW, 128), dtype=ml_dtypes.bfloat16)
        np.put_along_axis(ohtab, rel_i[:, :, None],
                          nrm_c[:, :, None].astype(ml_dtypes.bfloat16), axis=2)

        # per-node columns
        d2 = np.zeros((128, NW), dtype=np.float32)
        ln = np.arange(sizes[c], dtype=np.int64)
        d2[ln % 128, ln // 128] = dinv2[bounds[c] + ln]
        pool = np.zeros((128, NW * GPC), dtype=ml_dtypes.bfloat16)
        g_of = batch[bounds[c] + ln].astype(np.int64) - c * GPC
        pool[ln % 128, (ln // 128) * GPC + g_of] = 1.0

        x_nm = np.zeros((S, F), dtype=ml_dtypes.bfloat16)
        x_nm[: sizes[c]] = x[bounds[c]: bounds[c + 1]].astype(ml_dtypes.bfloat16)

        m = {"x_nm": x_nm, "gidx": gidx, "ohtab": ohtab.reshape(128, -1),
             "dinv2": d2, "pool": pool}
        m.update(shared)
        in_maps.append(m)
    return in_maps


def _build_program(stage=99):
    nc = bacc.Bacc("TRN2", target_bir_lowering=False, debug=False,
                   num_devices=NC, num_swdge_queues=4)
    AF = mybir.ActivationFunctionType
    OP = mybir.AluOpType

    ap = {}
    for name, shape, dt in [
        ("x_nm", [S, F], bf16), ("xtab", [TAB, F], bf16),
        ("gidx", [128, NCH * NBAT * ICOLS], i16),
        ("ohtab", [128, NW * CPW * 128], bf16),
        ("dinv2", [128, NW], f32), ("pidx", [128, 1], f32),
        ("pool", [128, NW * GPC], bf16),
        ("iota", [128, 128], bf16), ("identf", [128, 128], f32),
        ("identb", [128, 128], bf16),
        ("convw", [F, 6 * F], bf16), ("convb", [F, 6], f32),
        ("jkw", [F, 6 * F], bf16), ("jkb", [F, NB], f32),
        ("bns", [F, NB], f32), ("bnt", [F, NB], f32),
        ("l1w", [F, NB * F], f32), ("l1b", [F, 1], f32),
        ("l2w", [F, 10], f32), ("l2b", [10, 1], f32),
    ]:
        ap[name] = nc.dram_tensor(name, shape, dt, kind="ExternalInput").ap()
    out_ap = nc.dram_tensor("out", [GPC, 10], f32, kind="ExternalOutput").ap()

    with tile.TileContext(nc) as tc:
        with (
            tc.tile_pool(name="dram", bufs=1, space="DRAM") as dram,
            tc.tile_pool(name="pers", bufs=1) as pers,
            tc.tile_pool(name="rot", bufs=1) as rot,
            tc.tile_pool(name="psum", bufs=1, space="PSUM") as psum,
        ):
            ag_in = dram.tile([S, F], bf16)

            # ---- persistent SBUF loads
            sb = {}
            for name in ["gidx", "dinv2", "pidx", "pool",
                         "iota", "identf", "identb", "convw", "convb", "jkw",
                         "jkb", "bns", "bnt", "l1w", "l1b", "l2w", "l2b"]:
                t_ = pers.tile(list(ap[name].shape), ap[name].dtype, name=f"sb_{name}")
                nc.sync.dma_start(t_[:], ap[name][:])
                sb[name] = t_

            h_nm = pers.tile([128, NW, F], bf16, name="h_nm")
            h1_fm = pers.tile([128, S], bf16, name="h1_fm")
            h2_fm = pers.tile([128, S], bf16, name="h2_fm")
            hb_fm = pers.tile([128, S], bf16, name="hb_fm")
            z_sb = pers.tile([128, NB, GPC], f32, name="z_sb")
            agg_sb = pers.tile([128, NW, F], f32, name="agg_sb")
            diag = pers.tile([128, NW, 128], bf16, name="diag")

            # diag(dinv2) blocks, layer-invariant: diag[p, w, d] = (d==p)*dinv2
            for w in range(NW):
                nc.vector.tensor_scalar(
                    out=diag[:, w, :], in0=sb["iota"][:],
                    scalar1=sb["pidx"][:], scalar2=sb["dinv2"][:, w:w + 1],
                    op0=OP.is_equal, op1=OP.mult)

            # ohtab viewed as [128, NW, CPW*128]: per-(k,b) slice is
            # [128, WB windows, CH*128] with window stride CPW*128.
            oh3 = ap["ohtab"].rearrange("p (w x) -> p w x", w=NW)

            def conv(lk, tables, h_out, write_nm):
                for k in range(NCH):
                    for b in range(NBAT):
                        G = rot.tile([128, WB * CH, F], bf16, tag="G", bufs=3,
                                     name="G")
                        gc = (k * NBAT + b) * ICOLS
                        nc.gpsimd.dma_gather(
                            out_ap=G[:], in_ap=tables[k][:],
                            idxs_ap=sb["gidx"][:, gc:gc + ICOLS],
                            num_idxs=NIDX, num_idxs_reg=NIDX, elem_size=F,
                            single_packet=False,
                            queue_num=(k * NBAT + b) % 4)
                        oh = rot.tile([128, WB, CH, 128], bf16, tag="oh",
                                      bufs=2, name="oh")
                        nc.sync.dma_start(
                            oh[:],
                            oh3[:, b * WB:(b + 1) * WB,
                                k * CH * 128:(k + 1) * CH * 128]
                            .rearrange("p w (c d) -> p w c d", c=CH))
                        for wi in range(WB):
                            w = b * WB + wi
                            pp = psum.tile([128, F], f32, tag="pp", bufs=4,
                                           name="pp")
                            if k == 0:
                                # self-loop: agg[f,d] += h[d,f]*dinv2[d]
                                nc.tensor.matmul(pp[:], h_nm[:, w, :],
                                                 diag[:, w, :],
                                                 start=True, stop=False)
                            for c in range(CH):
                                nc.tensor.matmul(
                                    pp[:], G[:, wi * CH + c, :], oh[:, wi, c, :],
                                    start=(k != 0 and c == 0),
                                    stop=(c == CH - 1))
                            if k == 0:
                                nc.vector.tensor_copy(agg_sb[:, w, :], pp[:])
                            else:
                                nc.vector.tensor_tensor(
                                    out=agg_sb[:, w, :], in0=agg_sb[:, w, :],
                                    in1=pp[:], op=OP.add)
                                # tail: W matmul + bias + relu (feat-major)
                                tTs = rot.tile([128, F], bf16, tag="tTs",
                                               bufs=3, name="tTs")
                                nc.scalar.copy(tTs[:], agg_sb[:, w, :])
                                hn = psum.tile([128, F], f32, tag="hn", bufs=2,
                                               name="hn")
                                nc.tensor.matmul(
                                    hn[:], sb["convw"][:, lk * F:(lk + 1) * F],
                                    tTs[:], start=True, stop=True)
                                nc.scalar.activation(
                                    h_out[:, w * 128:(w + 1) * 128], hn[:],
                                    AF.Relu, bias=sb["convb"][:, lk:lk + 1])
                                if write_nm:
                                    hnT = psum.tile([128, F], bf16, tag="hnT",
                                                    bufs=1, name="hnT")
                                    nc.tensor.transpose(
                                        hnT[:], h_out[:, w * 128:(w + 1) * 128],
                                        sb["identb"][:])
                                    nc.scalar.copy(h_nm[:, w, :], hnT[:])
                        if k == NCH - 1 and write_nm:
                            r0 = b * WB * 128
                            nc.sync.dma_start(
                                ag_in[r0:r0 + WB * 128, :]
                                .rearrange("(w p) f -> p w f", p=128),
                                h_nm[:, b * WB:(b + 1) * WB, :])

            def allgather(i):
                tabs = []
                for k in range(NCH):
                    tk = dram.tile([CHROWS, F], bf16, addr_space="Shared",
                                   tag=f"t{_rep[0]}_{i}_{k}",
                                   name=f"t{_rep[0]}_{i}_{k}")
                    nc.gpsimd.collective_compute(
                        "AllGather", OP.bypass,
                        replica_groups=[list(range(NC))],
                        ins=[ag_in[k * CHS:(k + 1) * CHS, :].opt()],
                        outs=[tk.opt()])
                    tabs.append(tk)
                return tabs

            def jk(li, last):
                pooled = psum.tile([128, GPC], f32, tag="pooled", bufs=1,
                                   name="pooled")
                for w in range(NW):
                    hb = psum.tile([128, F], f32, tag="hn", bufs=2, name="hb")
                    nc.tensor.matmul(hb[:], sb["jkw"][:, (2 * li) * F:(2 * li + 1) * F],
                                     h1_fm[:, w * 128:(w + 1) * 128],
                                     start=True, stop=False)
                    nc.tensor.matmul(hb[:], sb["jkw"][:, (2 * li + 1) * F:(2 * li + 2) * F],
                                     h2_fm[:, w * 128:(w + 1) * 128],
                                     start=False, stop=True)
                    nc.scalar.activation(hb_fm[:, w * 128:(w + 1) * 128], hb[:],
                                         AF.Relu, bias=sb["jkb"][:, li:li + 1])
                    hnT = psum.tile([128, F], bf16, tag="hnT", bufs=1, name="hnT")
                    nc.tensor.transpose(hnT[:], hb_fm[:, w * 128:(w + 1) * 128],
                                        sb["identb"][:])
                    nc.scalar.copy(h_nm[:, w, :], hnT[:])
                    if not last and (w % WB == WB - 1):
                        r0 = (w - WB + 1) * 128
                        nc.sync.dma_start(
                            ag_in[r0:r0 + WB * 128, :]
                            .rearrange("(w p) f -> p w f", p=128),
                            h_nm[:, w - WB + 1:w + 1, :])
                    nc.tensor.matmul(pooled[:], h_nm[:, w, :],
                                     sb["pool"][:, w * GPC:(w + 1) * GPC],
                                     start=(w == 0), stop=(w == NW - 1))
                nc.scalar.copy(z_sb[:, li, :], pooled[:])

            # ---- main flow
            _rep = [0]
            steps = [
                lambda: conv(0, [ap["xtab"][k * CHROWS:(k + 1) * CHROWS, :] for k in range(NCH)], h1_fm, True),
                lambda: allgather(0),
                lambda t: conv(1, t, h2_fm, False),
                lambda: jk(0, False),
                lambda: allgather(1),
                lambda t: conv(2, t, h1_fm, True),
                lambda: allgather(2),
                lambda t: conv(3, t, h2_fm, False),
                lambda: jk(1, False),
                lambda: allgather(3),
                lambda t: conv(4, t, h1_fm, True),
                lambda: allgather(4),
                lambda t: conv(5, t, h2_fm, False),
                lambda: jk(2, True),
            ]
            for rep in range(REPEAT):
                _rep[0] = rep
                for b in range(NBAT):
                    nc.sync.dma_start(
                        h_nm[:, b * WB:(b + 1) * WB, :],
                        ap["x_nm"][b * WB * 128:(b + 1) * WB * 128, :]
                        .rearrange("(w p) f -> p w f", p=128))
                table = None
                for i, st in enumerate(steps):
                    if i >= stage:
                        break
                    r = st(table) if st.__code__.co_argcount else st()
                    if r is not None:
                        table = r

            # ---- head
            if stage < 14:
                outt0 = rot.tile([GPC, 10], f32, tag="outt", bufs=1, name="outt0")
                nc.vector.tensor_copy(outt0[:], h1_fm[0:GPC, 0:10])
                nc.sync.dma_start(out_ap[:], outt0[:])
            else:
                _head(nc, tc, rot, psum, sb, z_sb, out_ap)

    nc.compile()
    return nc


def _head(nc, tc, rot, psum, sb, z_sb, out_ap):
    AF = mybir.ActivationFunctionType
    OP = mybir.AluOpType
    zbn = rot.tile([128, NB, GPC], f32, tag="zbn", bufs=1, name="zbn")
    for t in range(NB):
        nc.vector.tensor_scalar(
            out=zbn[:, t, :], in0=z_sb[:, t, :],
            scalar1=sb["bns"][:, t:t + 1], scalar2=sb["bnt"][:, t:t + 1],
            op0=OP.mult, op1=OP.add)
    a1 = psum.tile([128, GPC], f32, tag="hn", bufs=2, name="a1")
    for t in range(NB):
        nc.tensor.matmul(a1[:], sb["l1w"][:, t * F:(t + 1) * F],
                         zbn[:, t, :], start=(t == 0), stop=(t == NB - 1))
    a1s = rot.tile([128, GPC], f32, tag="a1s", bufs=1, name="a1s")
    nc.scalar.activation(a1s[:], a1[:], AF.Relu, bias=sb["l1b"][:])
    z2 = psum.tile([10, GPC], f32, tag="pooled", bufs=1, name="z2")
    nc.tensor.matmul(z2[:], sb["l2w"][:], a1s[:], start=True, stop=True)
    z2s = rot.tile([10, GPC], f32, tag="z2s", bufs=1, name="z2s")
    nc.scalar.activation(z2s[:], z2[:], AF.Identity, bias=sb["l2b"][:])
    z2T = psum.tile([GPC, 10], f32, tag="hnT", bufs=1, name="z2T")
    nc.tensor.transpose(z2T[:], z2s[:], sb["identf"][0:10, 0:10])
    z2Ts = rot.tile([GPC, 10], f32, tag="z2Ts", bufs=1, name="z2Ts")
    nc.vector.tensor_copy(z2Ts[:], z2T[:])
    negm = rot.tile([GPC, 1], f32, tag="negm", bufs=1, name="negm")
    nc.vector.tensor_reduce(negm[:], z2Ts[:], mybir.AxisListType.X,
                            OP.max, negate=True)
    et = rot.tile([GPC, 10], f32, tag="et", bufs=1, name="et")
    nc.scalar.activation(et[:], z2Ts[:], AF.Exp, bias=negm[:])
    ssum = rot.tile([GPC, 1], f32, tag="ssum", bufs=1, name="ssum")
    nc.vector.tensor_reduce(ssum[:], et[:], mybir.AxisListType.X, OP.add)
    rcp = rot.tile([GPC, 1], f32, tag="rcp", bufs=1, name="rcp")
    nc.vector.reciprocal(rcp[:], ssum[:])
    outt = rot.tile([GPC, 10], f32, tag="outt", bufs=1, name="outt")
    nc.vector.tensor_scalar_mul(outt[:], et[:], rcp[:])
    nc.sync.dma_start(out_ap[:], outt[:])


def _get_program():
    global _PROGRAM
    if _PROGRAM is None:
        _PROGRAM = _build_program()
    return _PROGRAM


def kernel(**inputs) -> np.ndarray:
    in_maps = _preprocess(inputs)
    nc = _get_program()
    res = run_bass_kernel_spmd(nc, in_maps, list(range(NC)))
    return np.concatenate([res.results[c]["out"] for c in range(NC)], axis=0)


# revision 14
# speedup vs baseline: 1.6100x; 1.0883x over previous
"""Trainium2 Bass kernel for nn_BaseModel_14499809591724 (GNN message passing).

Strategy (8 NeuronCores, data-parallel over graph batches):
  - Nodes are split into 8 contiguous shards at graph boundaries (batch is
    sorted), padded to S=6400 rows each; full node table = [8*S, 128] bf16.
  - Each core owns the edges whose dst falls in its shard. Edges are sorted by
    (dst window of 128 nodes, src-table chunk) and chunked into groups of 128.
  - Per GCN conv: batched dma_gather of h[src] rows (WB windows per call) from
    the replicated DRAM table; scaled one-hots for a whole (window, chunk) are
    built with TWO wide DVE tensor_tensor ops using broadcast (stride-0) APs;
    scatter-reduce via PE matmul with the GATHERED rows stationary, producing
    feat-major agg directly (no transpose); self-loop term is one extra matmul
    against a precomputed diag(dinv2) block; then W + bias + ReLU.
  - After each conv that feeds another conv, the 8 local shards are AllGathered
    (bf16, 2 chunks for progressive overlap) to rebuild the replicated table.
  - JumpingKnowledge + per-graph pooling (one-hot matmul) + BN + MLP head +
    softmax run per core on its own 64 graphs; host concatenates 8 x [64, 10].

All heavy compute runs on device. Host does index/layout preprocessing and
edge-weight normalization (deg/dinv/norm), which is sharding metadata.
"""
import sys
import numpy as np
import ml_dtypes

sys.path.insert(0, "/opt/trn_rl_repo")

from concourse import bacc, tile, mybir  # noqa: E402
from concourse.bass_utils import run_bass_kernel_spmd  # noqa: E402

# ---- model / sharding constants (shapes fixed by the problem) ----
NC = 8
N_NODES = 50000
N_EDGES = 800000
F = 128
B = 512
GPC = B // NC          # graphs per core = 64
S = 6400               # padded nodes per shard (max real shard is 6368)
NW = S // 128          # 50 windows per core
TAB = NC * S           # 51200 table rows
NCH = 2                # table chunks (progressive AllGather pipeline)
CHS = S // NCH         # 3200 shard rows per chunk
CHROWS = NC * CHS      # 25600 table rows per chunk (int16-safe)
CH = 9                 # 128-edge groups per (window, table-chunk); max seen 1112
CPW = NCH * CH         # 18 one-hot columns per window
WB = 5                 # windows per dma_gather batch
NBAT = NW // WB        # 10 gather batches per (conv, chunk)
NIDX = WB * CH * 128   # 5760 idxs per gather
ICOLS = NIDX // 16     # 360 wrapped idx columns per gather
NB = 3
BN_EPS = 1e-5

f32 = mybir.dt.float32
bf16 = mybir.dt.bfloat16
fp8 = mybir.dt.float8e4
i16 = mybir.dt.int16

_PROGRAM = None
import os
REPEAT = int(os.environ.get("REPEAT", "1"))


def _wrap_idxs(runs: np.ndarray) -> np.ndarray:
    """[R, NIDX] int -> [128, R*ICOLS] int16 (16-partition wrap, 8x replicated)."""
    r = runs.shape[0]
    w = runs.reshape(r, -1, 16).transpose(2, 0, 1).reshape(16, -1)
    return np.tile(w.astype(np.int16), (8, 1))


def _preprocess(inp: dict):
    batch = np.asarray(inp["batch"])
    ei = np.asarray(inp["edge_index"])
    ew = np.asarray(inp["edge_attr"], dtype=np.float32)
    x = np.asarray(inp["x"], dtype=np.float32)
    src, dst = ei[0].astype(np.int64), ei[1].astype(np.int64)

    bounds = np.searchsorted(batch, np.arange(0, B + 1, GPC)).astype(np.int64)
    sizes = np.diff(bounds)
    assert sizes.max() <= S, f"shard overflow: {sizes.max()} > {S}"

    node = np.arange(N_NODES, dtype=np.int64)
    core_of = (np.searchsorted(bounds, node, side="right") - 1).astype(np.int64)
    off = node - bounds[core_of]
    # chunk-major table: row = chunk*CHROWS + core*CHS + (off % CHS)
    tab = (off // CHS) * CHROWS + core_of * CHS + (off % CHS)

    deg = (np.bincount(dst, weights=ew.astype(np.float64), minlength=N_NODES) + 1.0)
    deg = deg.astype(np.float32)
    dinv = 1.0 / np.sqrt(deg)
    norm = (dinv[src] * ew * dinv[dst]).astype(np.float32)
    dinv2 = (1.0 / deg).astype(np.float32)

    # full replicated x table (node-major, bf16)
    xtab = np.zeros((TAB, F), dtype=ml_dtypes.bfloat16)
    xtab[tab] = x.astype(ml_dtypes.bfloat16)

    iota = np.tile(np.arange(128, dtype=np.float32), (128, 1)).astype(ml_dtypes.bfloat16)
    identf = np.eye(128, dtype=np.float32)
    identb = np.eye(128, dtype=ml_dtypes.bfloat16)
    pidx = np.arange(128, dtype=np.float32).reshape(128, 1)

    # weights
    conv_w = np.asarray(inp["conv_w"], dtype=np.float32).reshape(6, F, F)
    convw = conv_w.transpose(1, 0, 2).reshape(F, 6 * F).astype(ml_dtypes.bfloat16)
    convb = np.asarray(inp["conv_b"], dtype=np.float32).reshape(6, F).T.copy()
    jk_w = np.asarray(inp["jk_w"], dtype=np.float32).reshape(NB, 2, F, F).reshape(6, F, F)
    jkw = jk_w.transpose(1, 0, 2).reshape(F, 6 * F).astype(ml_dtypes.bfloat16)
    jkb = np.asarray(inp["jk_b"], dtype=np.float32).T.copy()
    s = (np.asarray(inp["bn_gamma"], dtype=np.float32)
         / np.sqrt(np.asarray(inp["bn_var"], dtype=np.float32) + BN_EPS))
    t = (np.asarray(inp["bn_beta"], dtype=np.float32)
         - np.asarray(inp["bn_mean"], dtype=np.float32) * s)
    bns = s.reshape(NB, F).T.copy()
    bnt = t.reshape(NB, F).T.copy()
    lin1_w = np.asarray(inp["lin1_w"], dtype=np.float32).reshape(NB, F, F)
    l1w = lin1_w.transpose(1, 0, 2).reshape(F, NB * F).copy()
    l1b = np.asarray(inp["lin1_b"], dtype=np.float32).reshape(F, 1).copy()
    l2w = np.asarray(inp["lin2_w"], dtype=np.float32).copy()
    l2b = np.asarray(inp["lin2_b"], dtype=np.float32).reshape(10, 1).copy()

    shared = {
        "iota": iota, "identf": identf, "identb": identb, "pidx": pidx,
        "convw": convw, "convb": convb, "jkw": jkw, "jkb": jkb,
        "bns": bns, "bnt": bnt, "l1w": l1w, "l1b": l1b, "l2w": l2w, "l2b": l2b,
        "xtab": xtab,
    }

    dst_core = core_of[dst]
    dst_off = off[dst]
    src_tab = tab[src]

    in_maps = []
    for c in range(NC):
        eidx = np.flatnonzero(dst_core == c)
        e_win = dst_off[eidx] // 128
        e_k = src_tab[eidx] // CHROWS
        key = e_win * NCH + e_k
        order = np.argsort(key, kind="stable")
        eidx = eidx[order]
        key = key[order]
        counts = np.bincount(key, minlength=NW * NCH)
        assert (counts <= CH * 128).all(), f"chunk overflow core {c}"
        starts = np.concatenate([[0], np.cumsum(counts)])[:-1]
        pos = np.arange(len(eidx)) - starts[key]
        # slot space: [NW, NCH, CH, 128]
        slot = key * (CH * 128) + pos

        idx_slots = np.zeros(NW * NCH * CH * 128, dtype=np.int64)
        rel_slots = np.zeros(NW * NCH * CH * 128, dtype=np.float32)
        nrm_slots = np.zeros(NW * NCH * CH * 128, dtype=np.float32)
        idx_slots[slot] = src_tab[eidx] % CHROWS
        rel_slots[slot] = (dst_off[eidx] % 128).astype(np.float32)
        nrm_slots[slot] = norm[eidx]
        # empty slots: rel stays 0 but norm is 0, so one-hot row is all-zero.

        # gather idx runs, batched WB windows per gather: [NCH, NBAT, NIDX]
        runs = (idx_slots.reshape(NW, NCH, CH * 128)
                .transpose(1, 0, 2).reshape(NCH, NBAT, NIDX))
        gidx = _wrap_idxs(runs.reshape(NCH * NBAT, NIDX))  # [128, NCH*NBAT*ICOLS]
        # precomputed scaled one-hots (layer-invariant): [128, NW*CPW, 128]
        rel_i = rel_slots.reshape(NW * NCH * CH, 128).T.astype(np.int64)
        nrm_c = nrm_slots.reshape(NW * NCH * CH, 128).T
        ohtab = np.zeros((128, NW * CPW, 128), dtype=ml_dtypes.float8_e4m3)
        np.put_along_axis(ohtab, rel_i[:, :, None],
                          nrm_c[:, :, None].astype(ml_dtypes.float8_e4m3), axis=2)

        # per-node columns
        d2 = np.zeros((128, NW), dtype=np.float32)
        ln = np.arange(sizes[c], dtype=np.int64)
        d2[ln % 128, ln // 128] = dinv2[bounds[c] + ln]
        pool = np.zeros((128, NW * GPC), dtype=ml_dtypes.bfloat16)
        g_of = batch[bounds[c] + ln].astype(np.int64) - c * GPC
        pool[ln % 128, (ln // 128) * GPC + g_of] = 1.0

        x_nm = np.zeros((S, F), dtype=ml_dtypes.bfloat16)
        x_nm[: sizes[c]] = x[bounds[c]: bounds[c + 1]].astype(ml_dtypes.bfloat16)

        m = {"x_nm": x_nm, "gidx": gidx, "ohtab": ohtab.reshape(128, -1),
             "dinv2": d2, "pool": pool}
        m.update(shared)
        in_maps.append(m)
    return in_maps


def _build_program(stage=99):
    nc = bacc.Bacc("TRN2", target_bir_lowering=False, debug=False,
                   num_devices=NC, num_swdge_queues=4)
    AF = mybir.ActivationFunctionType
    OP = mybir.AluOpType

    ap = {}
    for name, shape, dt in [
        ("x_nm", [S, F], bf16), ("xtab", [TAB, F], bf16),
        ("gidx", [128, NCH * NBAT * ICOLS], i16),
        ("ohtab", [128, NW * CPW * 128], fp8),
        ("dinv2", [128, NW], f32), ("pidx", [128, 1], f32),
        ("pool", [128, NW * GPC], bf16),
        ("iota", [128, 128], bf16), ("identf", [128, 128], f32),
        ("identb", [128, 128], bf16),
        ("convw", [F, 6 * F], bf16), ("convb", [F, 6], f32),
        ("jkw", [F, 6 * F], bf16), ("jkb", [F, NB], f32),
        ("bns", [F, NB], f32), ("bnt", [F, NB], f32),
        ("l1w", [F, NB * F], f32), ("l1b", [F, 1], f32),
        ("l2w", [F, 10], f32), ("l2b", [10, 1], f32),
    ]:
        ap[name] = nc.dram_tensor(name, shape, dt, kind="ExternalInput").ap()
    out_ap = nc.dram_tensor("out", [GPC, 10], f32, kind="ExternalOutput").ap()

    with tile.TileContext(nc) as tc:
        with (
            tc.tile_pool(name="dram", bufs=1, space="DRAM") as dram,
            tc.tile_pool(name="pers", bufs=1) as pers,
            tc.tile_pool(name="rot", bufs=1) as rot,
            tc.tile_pool(name="psum", bufs=1, space="PSUM") as psum,
        ):
            ag_in = dram.tile([S, F], bf16)

            # ---- persistent SBUF loads
            sb = {}
            for name in ["gidx", "dinv2", "pidx", "pool",
                         "iota", "identf", "identb", "convw", "convb", "jkw",
                         "jkb", "bns", "bnt", "l1w", "l1b", "l2w", "l2b"]:
                t_ = pers.tile(list(ap[name].shape), ap[name].dtype, name=f"sb_{name}")
                nc.sync.dma_start(t_[:], ap[name][:])
                sb[name] = t_

            h_nm = pers.tile([128, NW, F], bf16, name="h_nm")
            h1_fm = pers.tile([128, S], bf16, name="h1_fm")
            h2_fm = pers.tile([128, S], bf16, name="h2_fm")
            hb_fm = pers.tile([128, S], bf16, name="hb_fm")
            z_sb = pers.tile([128, NB, GPC], f32, name="z_sb")
            agg_sb = pers.tile([128, NW, F], f32, name="agg_sb")
            diag = pers.tile([128, NW, 128], bf16, name="diag")

            # diag(dinv2) blocks, layer-invariant: diag[p, w, d] = (d==p)*dinv2
            for w in range(NW):
                nc.vector.tensor_scalar(
                    out=diag[:, w, :], in0=sb["iota"][:],
                    scalar1=sb["pidx"][:], scalar2=sb["dinv2"][:, w:w + 1],
                    op0=OP.is_equal, op1=OP.mult)

            # ohtab viewed as [128, NW, CPW*128]: per-(k,b) slice is
            # [128, WB windows, CH*128] with window stride CPW*128.
            oh3 = ap["ohtab"].rearrange("p (w x) -> p w x", w=NW)

            def conv(lk, tables, h_out, write_nm):
                for k in range(NCH):
                    for b in range(NBAT):
                        G = rot.tile([128, WB * CH, F], bf16, tag="G", bufs=3,
                                     name="G")
                        gc = (k * NBAT + b) * ICOLS
                        nc.gpsimd.dma_gather(
                            out_ap=G[:], in_ap=tables[k][:],
                            idxs_ap=sb["gidx"][:, gc:gc + ICOLS],
                            num_idxs=NIDX, num_idxs_reg=NIDX, elem_size=F,
                            single_packet=False,
                            queue_num=(k * NBAT + b) % 4)
                        oh = rot.tile([128, WB, CH, 128], fp8, tag="oh",
                                      bufs=3, name="oh")
                        nc.sync.dma_start(
                            oh[:],
                            oh3[:, b * WB:(b + 1) * WB,
                                k * CH * 128:(k + 1) * CH * 128]
                            .rearrange("p w (c d) -> p w c d", c=CH))
                        for wi in range(WB):
                            w = b * WB + wi
                            pp = psum.tile([128, F], f32, tag="pp", bufs=4,
                                           name="pp")
                            if k == 0:
                                # self-loop: agg[f,d] += h[d,f]*dinv2[d]
                                nc.tensor.matmul(pp[:], h_nm[:, w, :],
                                                 diag[:, w, :],
                                                 start=True, stop=False)
                            for c in range(CH):
                                nc.tensor.matmul(
                                    pp[:], G[:, wi * CH + c, :], oh[:, wi, c, :],
                                    start=(k != 0 and c == 0),
                                    stop=(c == CH - 1))
                            if k == 0:
                                nc.vector.tensor_copy(agg_sb[:, w, :], pp[:])
                            else:
                                nc.vector.tensor_tensor(
                                    out=agg_sb[:, w, :], in0=agg_sb[:, w, :],
                                    in1=pp[:], op=OP.add)
                                # tail: W matmul + bias + relu (feat-major)
                                tTs = rot.tile([128, F], bf16, tag="tTs",
                                               bufs=3, name="tTs")
                                nc.scalar.copy(tTs[:], agg_sb[:, w, :])
                                hn = psum.tile([128, F], f32, tag="hn", bufs=2,
                                               name="hn")
                                nc.tensor.matmul(
                                    hn[:], sb["convw"][:, lk * F:(lk + 1) * F],
                                    tTs[:], start=True, stop=True)
                                nc.scalar.activation(
                                    h_out[:, w * 128:(w + 1) * 128], hn[:],
                                    AF.Relu, bias=sb["convb"][:, lk:lk + 1])
                                if write_nm:
                                    hnT = psum.tile([128, F], bf16, tag="hnT",
                                                    bufs=1, name="hnT")
                                    nc.tensor.transpose(
                                        hnT[:], h_out[:, w * 128:(w + 1) * 128],
                                        sb["identb"][:])
                                    nc.scalar.copy(h_nm[:, w, :], hnT[:])
                        if k == NCH - 1 and write_nm:
                            r0 = b * WB * 128
                            nc.sync.dma_start(
                                ag_in[r0:r0 + WB * 128, :]
                                .rearrange("(w p) f -> p w f", p=128),
                                h_nm[:, b * WB:(b + 1) * WB, :])

            def allgather(i):
                tabs = []
                for k in range(NCH):
                    tk = dram.tile([CHROWS, F], bf16, addr_space="Shared",
                                   tag=f"t{_rep[0]}_{i}_{k}",
                                   name=f"t{_rep[0]}_{i}_{k}")
                    nc.gpsimd.collective_compute(
                        "AllGather", OP.bypass,
                        replica_groups=[list(range(NC))],
                        ins=[ag_in[k * CHS:(k + 1) * CHS, :].opt()],
                        outs=[tk.opt()])
                    tabs.append(tk)
                return tabs

            def jk(li, last):
                pooled = psum.tile([128, GPC], f32, tag="pooled", bufs=1,
                                   name="pooled")
                for w in range(NW):
                    hb = psum.tile([128, F], f32, tag="hn", bufs=2, name="hb")
                    nc.tensor.matmul(hb[:], sb["jkw"][:, (2 * li) * F:(2 * li + 1) * F],
                                     h1_fm[:, w * 128:(w + 1) * 128],
                                     start=True, stop=False)
                    nc.tensor.matmul(hb[:], sb["jkw"][:, (2 * li + 1) * F:(2 * li + 2) * F],
                                     h2_fm[:, w * 128:(w + 1) * 128],
                                     start=False, stop=True)
                    nc.scalar.activation(hb_fm[:, w * 128:(w + 1) * 128], hb[:],
                                         AF.Relu, bias=sb["jkb"][:, li:li + 1])
                    hnT = psum.tile([128, F], bf16, tag="hnT", bufs=1, name="hnT")
                    nc.tensor.transpose(hnT[:], hb_fm[:, w * 128:(w + 1) * 128],
                                        sb["identb"][:])
                    nc.scalar.copy(h_nm[:, w, :], hnT[:])
                    if not last and (w % WB == WB - 1):
                        r0 = (w - WB + 1) * 128
                        nc.sync.dma_start(
                            ag_in[r0:r0 + WB * 128, :]
                            .rearrange("(w p) f -> p w f", p=128),
                            h_nm[:, w - WB + 1:w + 1, :])
                    nc.tensor.matmul(pooled[:], h_nm[:, w, :],
                                     sb["pool"][:, w * GPC:(w + 1) * GPC],
                                     start=(w == 0), stop=(w == NW - 1))
                nc.scalar.copy(z_sb[:, li, :], pooled[:])

            # ---- main flow
            _rep = [0]
            steps = [
                lambda: conv(0, [ap["xtab"][k * CHROWS:(k + 1) * CHROWS, :] for k in range(NCH)], h1_fm, True),
                lambda: allgather(0),
                lambda t: conv(1, t, h2_fm, False),
                lambda: jk(0, False),
                lambda: allgather(1),
                lambda t: conv(2, t, h1_fm, True),
                lambda: allgather(2),
                lambda t: conv(3, t, h2_fm, False),
                lambda: jk(1, False),
                lambda: allgather(3),
                lambda t: conv(4, t, h1_fm, True),
                lambda: allgather(4),
                lambda t: conv(5, t, h2_fm, False),
                lambda: jk(2, True),
            ]
            for rep in range(REPEAT):
                _rep[0] = rep
                for b in range(NBAT):
                    nc.sync.dma_start(
                        h_nm[:, b * WB:(b + 1) * WB, :],
                        ap["x_nm"][b * WB * 128:(b + 1) * WB * 128, :]
                        .rearrange("(w p) f -> p w f", p=128))
                table = None
                for i, st in enumerate(steps):
                    if i >= stage:
                        break
                    r = st(table) if st.__code__.co_argcount else st()
                    if r is not None:
                        table = r

            # ---- head
            if stage < 14:
                outt0 = rot.tile([GPC, 10], f32, tag="outt", bufs=1, name="outt0")
                nc.vector.tensor_copy(outt0[:], h1_fm[0:GPC, 0:10])
                nc.sync.dma_start(out_ap[:], outt0[:])
            else:
                _head(nc, tc, rot, psum, sb, z_sb, out_ap)

    nc.compile()
    return nc


def _head(nc, tc, rot, psum, sb, z_sb, out_ap):
    AF = mybir.ActivationFunctionType
    OP = mybir.AluOpType
    zbn = rot.tile([128, NB, GPC], f32, tag="zbn", bufs=1, name="zbn")
    for t in range(NB):
        nc.vector.tensor_scalar(
            out=zbn[:, t, :], in0=z_sb[:, t, :],
            scalar1=sb["bns"][:, t:t + 1], scalar2=sb["bnt"][:, t:t + 1],
            op0=OP.mult, op1=OP.add)
    a1 = psum.tile([128, GPC], f32, tag="hn", bufs=2, name="a1")
    for t in range(NB):
        nc.tensor.matmul(a1[:], sb["l1w"][:, t * F:(t + 1) * F],
                         zbn[:, t, :], start=(t == 0), stop=(t == NB - 1))
    a1s = rot.tile([128, GPC], f32, tag="a1s", bufs=1, name="a1s")
    nc.scalar.activation(a1s[:], a1[:], AF.Relu, bias=sb["l1b"][:])
    z2 = psum.tile([10, GPC], f32, tag="pooled", bufs=1, name="z2")
    nc.tensor.matmul(z2[:], sb["l2w"][:], a1s[:], start=True, stop=True)
    z2s = rot.tile([10, GPC], f32, tag="z2s", bufs=1, name="z2s")
    nc.scalar.activation(z2s[:], z2[:], AF.Identity, bias=sb["l2b"][:])
    z2T = psum.tile([GPC, 10], f32, tag="hnT", bufs=1, name="z2T")
    nc.tensor.transpose(z2T[:], z2s[:], sb["identf"][0:10, 0:10])
    z2Ts = rot.tile([GPC, 10], f32, tag="z2Ts", bufs=1, name="z2Ts")
    nc.vector.tensor_copy(z2Ts[:], z2T[:])
    negm = rot.tile([GPC, 1], f32, tag="negm", bufs=1, name="negm")
    nc.vector.tensor_reduce(negm[:], z2Ts[:], mybir.AxisListType.X,
                            OP.max, negate=True)
    et = rot.tile([GPC, 10], f32, tag="et", bufs=1, name="et")
    nc.scalar.activation(et[:], z2Ts[:], AF.Exp, bias=negm[:])
    ssum = rot.tile([GPC, 1], f32, tag="ssum", bufs=1, name="ssum")
    nc.vector.tensor_reduce(ssum[:], et[:], mybir.AxisListType.X, OP.add)
    rcp = rot.tile([GPC, 1], f32, tag="rcp", bufs=1, name="rcp")
    nc.vector.reciprocal(rcp[:], ssum[:])
    outt = rot.tile([GPC, 10], f32, tag="outt", bufs=1, name="outt")
    nc.vector.tensor_scalar_mul(outt[:], et[:], rcp[:])
    nc.sync.dma_start(out_ap[:], outt[:])


def _get_program():
    global _PROGRAM
    if _PROGRAM is None:
        _PROGRAM = _build_program()
    return _PROGRAM


def kernel(**inputs) -> np.ndarray:
    in_maps = _preprocess(inputs)
    nc = _get_program()
    res = run_bass_kernel_spmd(nc, in_maps, list(range(NC)))
    return np.concatenate([res.results[c]["out"] for c in range(NC)], axis=0)


# revision 23
# speedup vs baseline: 1.9060x; 1.1839x over previous
"""Trainium2 Bass kernel for nn_BaseModel_14499809591724 (GNN message passing).

Strategy (8 NeuronCores, data-parallel over graph batches):
  - Nodes are split into 8 contiguous shards at graph boundaries (batch is
    sorted), padded to S=6400 rows each; full node table = [8*S, 128] bf16.
  - Each core owns the edges whose dst falls in its shard. Edges are sorted by
    (dst window of 128 nodes, src-table chunk) and chunked into groups of 128.
  - Per GCN conv: batched dma_gather of h[src] rows (WB windows per call) from
    the replicated DRAM table; scaled one-hots for a whole (window, chunk) are
    built with TWO wide DVE tensor_tensor ops using broadcast (stride-0) APs;
    scatter-reduce via PE matmul with the GATHERED rows stationary, producing
    feat-major agg directly (no transpose); self-loop term is one extra matmul
    against a precomputed diag(dinv2) block; then W + bias + ReLU.
  - After each conv that feeds another conv, the 8 local shards are AllGathered
    (bf16, 2 chunks for progressive overlap) to rebuild the replicated table.
  - JumpingKnowledge + per-graph pooling (one-hot matmul) + BN + MLP head +
    softmax run per core on its own 64 graphs; host concatenates 8 x [64, 10].

All heavy compute runs on device. Host does index/layout preprocessing and
edge-weight normalization (deg/dinv/norm), which is sharding metadata.
"""
import sys
import numpy as np
import ml_dtypes

sys.path.insert(0, "/opt/trn_rl_repo")

from concourse import bacc, tile, mybir  # noqa: E402
from concourse.bass_utils import run_bass_kernel_spmd  # noqa: E402

# ---- model / sharding constants (shapes fixed by the problem) ----
NC = 8
N_NODES = 50000
N_EDGES = 800000
F = 128
B = 512
GPC = B // NC          # graphs per core = 64
S = 6400               # padded nodes per shard (max real shard is 6368)
NW = S // 128          # 50 windows per core
TAB = NC * S           # 51200 table rows
NCH = 2                # table chunks (progressive AllGather pipeline)
CHS = S // NCH         # 3200 shard rows per chunk
CHROWS = NC * CHS      # 25600 table rows per chunk (int16-safe)
CH = 9                 # 128-edge groups per (window, table-chunk); max seen 1112
CPW = NCH * CH         # 18 one-hot columns per window
WB = 5                 # windows per dma_gather batch
NBAT = NW // WB        # 10 gather batches per (conv, chunk)
NIDX = WB * CH * 128   # 5760 idxs per gather
ICOLS = NIDX // 16     # 360 wrapped idx columns per gather
NB = 3
BN_EPS = 1e-5

f32 = mybir.dt.float32
bf16 = mybir.dt.bfloat16
fp8 = mybir.dt.float8e4
i16 = mybir.dt.int16

_PROGRAM = None
import os
REPEAT = int(os.environ.get("REPEAT", "1"))
SKIP_AG = os.environ.get("SKIP_AG") == "1"    # don't emit collectives (timing probe)
XTAB_ALL = os.environ.get("XTAB_ALL") == "1"  # convs always gather xtab (timing probe)
SKIP_MM = os.environ.get("SKIP_MM") == "1"    # skip PE/DVE conv compute (timing probe)
SKIP_OH = os.environ.get("SKIP_OH") == "1"    # skip oh stream DMAs (timing probe)
SKIP_GATHER = os.environ.get("SKIP_GATHER") == "1"  # skip gathers (timing probe)


def _wrap_idxs(runs: np.ndarray) -> np.ndarray:
    """[R, NIDX] int -> [128, R*ICOLS] int16 (16-partition wrap, 8x replicated)."""
    r = runs.shape[0]
    w = runs.reshape(r, -1, 16).transpose(2, 0, 1).reshape(16, -1)
    return np.tile(w.astype(np.int16), (8, 1))


def _preprocess(inp: dict):
    batch = np.asarray(inp["batch"])
    ei = np.asarray(inp["edge_index"])
    ew = np.asarray(inp["edge_attr"], dtype=np.float32)
    x = np.asarray(inp["x"], dtype=np.float32)
    src, dst = ei[0].astype(np.int64), ei[1].astype(np.int64)

    bounds = np.searchsorted(batch, np.arange(0, B + 1, GPC)).astype(np.int64)
    sizes = np.diff(bounds)
    assert sizes.max() <= S, f"shard overflow: {sizes.max()} > {S}"

    node = np.arange(N_NODES, dtype=np.int64)
    core_of = (np.searchsorted(bounds, node, side="right") - 1).astype(np.int64)
    off = node - bounds[core_of]
    # chunk-major table: row = chunk*CHROWS + core*CHS + (off % CHS)
    tab = (off // CHS) * CHROWS + core_of * CHS + (off % CHS)

    deg = (np.bincount(dst, weights=ew.astype(np.float64), minlength=N_NODES) + 1.0)
    deg = deg.astype(np.float32)
    dinv = 1.0 / np.sqrt(deg)
    norm = (dinv[src] * ew * dinv[dst]).astype(np.float32)
    dinv2 = (1.0 / deg).astype(np.float32)

    # full replicated x table (node-major, bf16)
    xtab = np.zeros((TAB, F), dtype=ml_dtypes.bfloat16)
    xtab[tab] = x.astype(ml_dtypes.bfloat16)

    iota = np.tile(np.arange(128, dtype=np.float32), (128, 1)).astype(ml_dtypes.bfloat16)
    identf = np.eye(128, dtype=np.float32)
    identb = np.eye(128, dtype=ml_dtypes.bfloat16)
    pidx = np.arange(128, dtype=np.float32).reshape(128, 1)

    # weights
    conv_w = np.asarray(inp["conv_w"], dtype=np.float32).reshape(6, F, F)
    convw = conv_w.transpose(1, 0, 2).reshape(F, 6 * F).astype(ml_dtypes.bfloat16)
    convb = np.asarray(inp["conv_b"], dtype=np.float32).reshape(6, F).T.copy()
    jk_w = np.asarray(inp["jk_w"], dtype=np.float32).reshape(NB, 2, F, F).reshape(6, F, F)
    jkw = jk_w.transpose(1, 0, 2).reshape(F, 6 * F).astype(ml_dtypes.bfloat16)
    jkb = np.asarray(inp["jk_b"], dtype=np.float32).T.copy()
    s = (np.asarray(inp["bn_gamma"], dtype=np.float32)
         / np.sqrt(np.asarray(inp["bn_var"], dtype=np.float32) + BN_EPS))
    t = (np.asarray(inp["bn_beta"], dtype=np.float32)
         - np.asarray(inp["bn_mean"], dtype=np.float32) * s)
    bns = s.reshape(NB, F).T.copy()
    bnt = t.reshape(NB, F).T.copy()
    lin1_w = np.asarray(inp["lin1_w"], dtype=np.float32).reshape(NB, F, F)
    l1w = lin1_w.transpose(1, 0, 2).reshape(F, NB * F).copy()
    l1b = np.asarray(inp["lin1_b"], dtype=np.float32).reshape(F, 1).copy()
    l2w = np.asarray(inp["lin2_w"], dtype=np.float32).copy()
    l2b = np.asarray(inp["lin2_b"], dtype=np.float32).reshape(10, 1).copy()

    shared = {
        "iota": iota, "identf": identf, "identb": identb, "pidx": pidx,
        "convw": convw, "convb": convb, "jkw": jkw, "jkb": jkb,
        "bns": bns, "bnt": bnt, "l1w": l1w, "l1b": l1b, "l2w": l2w, "l2b": l2b,
        "xtab": xtab,
    }

    dst_core = core_of[dst]
    dst_off = off[dst]
    src_tab = tab[src]

    in_maps = []
    for c in range(NC):
        eidx = np.flatnonzero(dst_core == c)
        e_win = dst_off[eidx] // 128
        e_k = src_tab[eidx] // CHROWS
        key = e_win * NCH + e_k
        # sort by (group, src address): monotone DRAM addresses within each
        # group give the gather DMA row-buffer locality
        order = np.lexsort((src_tab[eidx], key))
        eidx = eidx[order]
        key = key[order]
        counts = np.bincount(key, minlength=NW * NCH)
        assert (counts <= CH * 128).all(), f"chunk overflow core {c}"
        starts = np.concatenate([[0], np.cumsum(counts)])[:-1]
        pos = np.arange(len(eidx)) - starts[key]
        # slot space: [NW, NCH, CH, 128]
        slot = key * (CH * 128) + pos

        idx_slots = np.zeros(NW * NCH * CH * 128, dtype=np.int64)
        rel_slots = np.zeros(NW * NCH * CH * 128, dtype=np.float32)
        nrm_slots = np.zeros(NW * NCH * CH * 128, dtype=np.float32)
        idx_slots[slot] = src_tab[eidx] % CHROWS
        rel_slots[slot] = (dst_off[eidx] % 128).astype(np.float32)
        nrm_slots[slot] = norm[eidx]
        # empty slots: rel stays 0 but norm is 0, so one-hot row is all-zero.

        # gather idx runs, batched WB windows per gather: [NCH, NBAT, NIDX]
        runs = (idx_slots.reshape(NW, NCH, CH * 128)
                .transpose(1, 0, 2).reshape(NCH, NBAT, NIDX))
        gidx = _wrap_idxs(runs.reshape(NCH * NBAT, NIDX))  # [128, NCH*NBAT*ICOLS]
        # precomputed scaled one-hots (layer-invariant): [128, NW*CPW, 128]
        rel_i = rel_slots.reshape(NW * NCH * CH, 128).T.astype(np.int64)
        nrm_c = nrm_slots.reshape(NW * NCH * CH, 128).T
        ohtab = np.zeros((128, NW * CPW, 128), dtype=ml_dtypes.float8_e4m3)
        np.put_along_axis(ohtab, rel_i[:, :, None],
                          nrm_c[:, :, None].astype(ml_dtypes.float8_e4m3), axis=2)

        # per-node columns
        d2 = np.zeros((128, NW), dtype=np.float32)
        ln = np.arange(sizes[c], dtype=np.int64)
        d2[ln % 128, ln // 128] = dinv2[bounds[c] + ln]
        pool = np.zeros((128, NW * GPC), dtype=ml_dtypes.bfloat16)
        g_of = batch[bounds[c] + ln].astype(np.int64) - c * GPC
        pool[ln % 128, (ln // 128) * GPC + g_of] = 1.0

        x_nm = np.zeros((S, F), dtype=ml_dtypes.bfloat16)
        x_nm[: sizes[c]] = x[bounds[c]: bounds[c + 1]].astype(ml_dtypes.bfloat16)

        m = {"x_nm": x_nm, "gidx": gidx, "ohtab": ohtab.reshape(128, -1),
             "dinv2": d2, "pool": pool}
        m.update(shared)
        in_maps.append(m)
    return in_maps


def _build_program(stage=99):
    nc = bacc.Bacc("TRN2", target_bir_lowering=False, debug=False,
                   num_devices=NC, num_swdge_queues=4)
    AF = mybir.ActivationFunctionType
    OP = mybir.AluOpType

    ap = {}
    for name, shape, dt in [
        ("x_nm", [S, F], bf16), ("xtab", [TAB, F], bf16),
        ("gidx", [128, NCH * NBAT * ICOLS], i16),
        ("ohtab", [128, NW * CPW * 128], fp8),
        ("dinv2", [128, NW], f32), ("pidx", [128, 1], f32),
        ("pool", [128, NW * GPC], bf16),
        ("iota", [128, 128], bf16), ("identf", [128, 128], f32),
        ("identb", [128, 128], bf16),
        ("convw", [F, 6 * F], bf16), ("convb", [F, 6], f32),
        ("jkw", [F, 6 * F], bf16), ("jkb", [F, NB], f32),
        ("bns", [F, NB], f32), ("bnt", [F, NB], f32),
        ("l1w", [F, NB * F], f32), ("l1b", [F, 1], f32),
        ("l2w", [F, 10], f32), ("l2b", [10, 1], f32),
    ]:
        ap[name] = nc.dram_tensor(name, shape, dt, kind="ExternalInput").ap()
    out_ap = nc.dram_tensor("out", [GPC, 10], f32, kind="ExternalOutput").ap()

    with tile.TileContext(nc) as tc:
        with (
            tc.tile_pool(name="dram", bufs=1, space="DRAM") as dram,
            tc.tile_pool(name="pers", bufs=1) as pers,
            tc.tile_pool(name="rot", bufs=1) as rot,
            tc.tile_pool(name="psum", bufs=1, space="PSUM") as psum,
        ):
            ag_in = dram.tile([S, F], bf16)

            # ---- persistent SBUF loads
            sb = {}
            for name in ["gidx", "dinv2", "pidx", "pool",
                         "iota", "identf", "identb", "convw", "convb", "jkw",
                         "jkb", "bns", "bnt", "l1w", "l1b", "l2w", "l2b"]:
                t_ = pers.tile(list(ap[name].shape), ap[name].dtype, name=f"sb_{name}")
                nc.sync.dma_start(t_[:], ap[name][:])
                sb[name] = t_

            h_nm = pers.tile([128, NW, F], bf16, name="h_nm")
            h1_fm = pers.tile([128, S], bf16, name="h1_fm")
            h2_fm = pers.tile([128, S], bf16, name="h2_fm")
            hb_fm = pers.tile([128, S], bf16, name="hb_fm")
            z_sb = pers.tile([128, NB, GPC], f32, name="z_sb")
            agg_sb = pers.tile([128, NW, F], f32, name="agg_sb")
            diag = pers.tile([128, NW, 128], bf16, name="diag")

            # diag(dinv2) blocks, layer-invariant: diag[p, w, d] = (d==p)*dinv2
            for w in range(NW):
                nc.vector.tensor_scalar(
                    out=diag[:, w, :], in0=sb["iota"][:],
                    scalar1=sb["pidx"][:], scalar2=sb["dinv2"][:, w:w + 1],
                    op0=OP.is_equal, op1=OP.mult)

            # ohtab viewed as [128, NW, CPW*128]: per-(k,b) slice is
            # [128, WB windows, CH*128] with window stride CPW*128.
            oh3 = ap["ohtab"].rearrange("p (w x) -> p w x", w=NW)

            def conv(lk, tables, h_out, write_nm):
                for k in range(NCH):
                    for b in range(NBAT):
                        G = rot.tile([128, WB * CH, F], bf16, tag="G", bufs=3,
                                     name="G")
                        gc = (k * NBAT + b) * ICOLS
                        if not SKIP_GATHER:
                            nc.gpsimd.dma_gather(
                                out_ap=G[:], in_ap=tables[k][:],
                                idxs_ap=sb["gidx"][:, gc:gc + ICOLS],
                                num_idxs=NIDX, num_idxs_reg=NIDX, elem_size=F,
                                single_packet=False,
                                queue_num=(k * NBAT + b) % 4)
                        oh = rot.tile([128, WB, CH, 128], fp8, tag="oh",
                                      bufs=3, name="oh")
                        if not SKIP_OH:
                            nc.sync.dma_start(
                                oh[:],
                                oh3[:, b * WB:(b + 1) * WB,
                                    k * CH * 128:(k + 1) * CH * 128]
                                .rearrange("p w (c d) -> p w c d", c=CH))
                        if SKIP_MM:
                            continue
                        for wi in range(WB):
                            w = b * WB + wi
                            pp = psum.tile([128, F], f32, tag="pp", bufs=4,
                                           name="pp")
                            if k == 0:
                                # self-loop: agg[f,d] += h[d,f]*dinv2[d]
                                nc.tensor.matmul(pp[:], h_nm[:, w, :],
                                                 diag[:, w, :],
                                                 start=True, stop=False)
                            for c in range(CH):
                                nc.tensor.matmul(
                                    pp[:], G[:, wi * CH + c, :], oh[:, wi, c, :],
                                    start=(k != 0 and c == 0),
                                    stop=(c == CH - 1))
                            if k == 0:
                                nc.vector.tensor_copy(agg_sb[:, w, :], pp[:])
                            else:
                                nc.vector.tensor_tensor(
                                    out=agg_sb[:, w, :], in0=agg_sb[:, w, :],
                                    in1=pp[:], op=OP.add)
                                # tail: W matmul + bias + relu (feat-major)
                                tTs = rot.tile([128, F], bf16, tag="tTs",
                                               bufs=3, name="tTs")
                                nc.scalar.copy(tTs[:], agg_sb[:, w, :])
                                hn = psum.tile([128, F], f32, tag="hn", bufs=2,
                                               name="hn")
                                nc.tensor.matmul(
                                    hn[:], sb["convw"][:, lk * F:(lk + 1) * F],
                                    tTs[:], start=True, stop=True)
                                nc.scalar.activation(
                                    h_out[:, w * 128:(w + 1) * 128], hn[:],
                                    AF.Relu, bias=sb["convb"][:, lk:lk + 1])
                                if write_nm:
                                    hnT = psum.tile([128, F], bf16, tag="hnT",
                                                    bufs=1, name="hnT")
                                    nc.tensor.transpose(
                                        hnT[:], h_out[:, w * 128:(w + 1) * 128],
                                        sb["identb"][:])
                                    nc.scalar.copy(h_nm[:, w, :], hnT[:])
                        if k == NCH - 1 and write_nm and not SKIP_MM:
                            r0 = b * WB * 128
                            nc.sync.dma_start(
                                ag_in[r0:r0 + WB * 128, :]
                                .rearrange("(w p) f -> p w f", p=128),
                                h_nm[:, b * WB:(b + 1) * WB, :])

            xtabs = [ap["xtab"][k * CHROWS:(k + 1) * CHROWS, :]
                     for k in range(NCH)]

            def allgather(i):
                if SKIP_AG:
                    return xtabs
                tabs = []
                for k in range(NCH):
                    tk = dram.tile([CHROWS, F], bf16, addr_space="Shared",
                                   tag=f"t{_rep[0]}_{i}_{k}",
                                   name=f"t{_rep[0]}_{i}_{k}")
                    nc.gpsimd.collective_compute(
                        "AllGather", OP.bypass,
                        replica_groups=[list(range(NC))],
                        ins=[ag_in[k * CHS:(k + 1) * CHS, :].opt()],
                        outs=[tk.opt()])
                    tabs.append(tk)
                return xtabs if XTAB_ALL else tabs

            def jk(li, last):
                if SKIP_MM:
                    return
                pooled = psum.tile([128, GPC], f32, tag="pooled", bufs=1,
                                   name="pooled")
                for w in range(NW):
                    hb = psum.tile([128, F], f32, tag="hn", bufs=2, name="hb")
                    nc.tensor.matmul(hb[:], sb["jkw"][:, (2 * li) * F:(2 * li + 1) * F],
                                     h1_fm[:, w * 128:(w + 1) * 128],
                                     start=True, stop=False)
                    nc.tensor.matmul(hb[:], sb["jkw"][:, (2 * li + 1) * F:(2 * li + 2) * F],
                                     h2_fm[:, w * 128:(w + 1) * 128],
                                     start=False, stop=True)
                    nc.scalar.activation(hb_fm[:, w * 128:(w + 1) * 128], hb[:],
                                         AF.Relu, bias=sb["jkb"][:, li:li + 1])
                    hnT = psum.tile([128, F], bf16, tag="hnT", bufs=1, name="hnT")
                    nc.tensor.transpose(hnT[:], hb_fm[:, w * 128:(w + 1) * 128],
                                        sb["identb"][:])
                    nc.scalar.copy(h_nm[:, w, :], hnT[:])
                    if not last and (w % WB == WB - 1):
                        r0 = (w - WB + 1) * 128
                        nc.sync.dma_start(
                            ag_in[r0:r0 + WB * 128, :]
                            .rearrange("(w p) f -> p w f", p=128),
                            h_nm[:, w - WB + 1:w + 1, :])
                    nc.tensor.matmul(pooled[:], h_nm[:, w, :],
                                     sb["pool"][:, w * GPC:(w + 1) * GPC],
                                     start=(w == 0), stop=(w == NW - 1))
                nc.scalar.copy(z_sb[:, li, :], pooled[:])

            # ---- main flow
            _rep = [0]
            steps = [
                lambda: conv(0, xtabs, h1_fm, True),
                lambda: allgather(0),
                lambda t: conv(1, t, h2_fm, False),
                lambda: jk(0, False),
                lambda: allgather(1),
                lambda t: conv(2, t, h1_fm, True),
                lambda: allgather(2),
                lambda t: conv(3, t, h2_fm, False),
                lambda: jk(1, False),
                lambda: allgather(3),
                lambda t: conv(4, t, h1_fm, True),
                lambda: allgather(4),
                lambda t: conv(5, t, h2_fm, False),
                lambda: jk(2, True),
            ]
            for rep in range(REPEAT):
                _rep[0] = rep
                for b in range(NBAT):
                    nc.sync.dma_start(
                        h_nm[:, b * WB:(b + 1) * WB, :],
                        ap["x_nm"][b * WB * 128:(b + 1) * WB * 128, :]
                        .rearrange("(w p) f -> p w f", p=128))
                table = None
                for i, st in enumerate(steps):
                    if i >= stage:
                        break
                    r = st(table) if st.__code__.co_argcount else st()
                    if r is not None:
                        table = r

            # ---- head
            if SKIP_MM:
                outt0 = rot.tile([GPC, 10], f32, tag="outt", bufs=1, name="outt0")
                nc.vector.tensor_copy(outt0[:], sb["identf"][0:GPC, 0:10])
                nc.sync.dma_start(out_ap[:], outt0[:])
            elif stage < 14:
                outt0 = rot.tile([GPC, 10], f32, tag="outt", bufs=1, name="outt0")
                nc.vector.tensor_copy(outt0[:], h1_fm[0:GPC, 0:10])
                nc.sync.dma_start(out_ap[:], outt0[:])
            else:
                _head(nc, tc, rot, psum, sb, z_sb, out_ap)

    nc.compile()
    return nc


def _head(nc, tc, rot, psum, sb, z_sb, out_ap):
    AF = mybir.ActivationFunctionType
    OP = mybir.AluOpType
    zbn = rot.tile([128, NB, GPC], f32, tag="zbn", bufs=1, name="zbn")
    for t in range(NB):
        nc.vector.tensor_scalar(
            out=zbn[:, t, :], in0=z_sb[:, t, :],
            scalar1=sb["bns"][:, t:t + 1], scalar2=sb["bnt"][:, t:t + 1],
            op0=OP.mult, op1=OP.add)
    a1 = psum.tile([128, GPC], f32, tag="hn", bufs=2, name="a1")
    for t in range(NB):
        nc.tensor.matmul(a1[:], sb["l1w"][:, t * F:(t + 1) * F],
                         zbn[:, t, :], start=(t == 0), stop=(t == NB - 1))
    a1s = rot.tile([128, GPC], f32, tag="a1s", bufs=1, name="a1s")
    nc.scalar.activation(a1s[:], a1[:], AF.Relu, bias=sb["l1b"][:])
    z2 = psum.tile([10, GPC], f32, tag="pooled", bufs=1, name="z2")
    nc.tensor.matmul(z2[:], sb["l2w"][:], a1s[:], start=True, stop=True)
    z2s = rot.tile([10, GPC], f32, tag="z2s", bufs=1, name="z2s")
    nc.scalar.activation(z2s[:], z2[:], AF.Identity, bias=sb["l2b"][:])
    z2T = psum.tile([GPC, 10], f32, tag="hnT", bufs=1, name="z2T")
    nc.tensor.transpose(z2T[:], z2s[:], sb["identf"][0:10, 0:10])
    z2Ts = rot.tile([GPC, 10], f32, tag="z2Ts", bufs=1, name="z2Ts")
    nc.vector.tensor_copy(z2Ts[:], z2T[:])
    negm = rot.tile([GPC, 1], f32, tag="negm", bufs=1, name="negm")
    nc.vector.tensor_reduce(negm[:], z2Ts[:], mybir.AxisListType.X,
                            OP.max, negate=True)
    et = rot.tile([GPC, 10], f32, tag="et", bufs=1, name="et")
    nc.scalar.activation(et[:], z2Ts[:], AF.Exp, bias=negm[:])
    ssum = rot.tile([GPC, 1], f32, tag="ssum", bufs=1, name="ssum")
    nc.vector.tensor_reduce(ssum[:], et[:], mybir.AxisListType.X, OP.add)
    rcp = rot.tile([GPC, 1], f32, tag="rcp", bufs=1, name="rcp")
    nc.vector.reciprocal(rcp[:], ssum[:])
    outt = rot.tile([GPC, 10], f32, tag="outt", bufs=1, name="outt")
    nc.vector.tensor_scalar_mul(outt[:], et[:], rcp[:])
    nc.sync.dma_start(out_ap[:], outt[:])


def _get_program():
    global _PROGRAM
    if _PROGRAM is None:
        _PROGRAM = _build_program()
    return _PROGRAM


def kernel(**inputs) -> np.ndarray:
    in_maps = _preprocess(inputs)
    nc = _get_program()
    res = run_bass_kernel_spmd(nc, in_maps, list(range(NC)))
    return np.concatenate([res.results[c]["out"] for c in range(NC)], axis=0)
